# revision 46
# baseline (speedup 1.0000x reference)
"""LATTE GNN forward on 8 Trainium2 NeuronCores — v3.

Math (same collapse as baseline): per-edge message is v[dst], and the
segment-softmax weights over each dst's incoming edges sum to 1, so
    h_m[n] = v[n] * mask_m[n],  mask_m[n] = [n has an incoming edge in rel m]
    v      = feat @ Wr + br
    vl[n,h] = v[n,h,:].rel_attn_l[h]   (folded: feat @ (Wr @ RLbd))
    vr[n,h] = v[n,h,:].rel_attn_r[h]
    logit[n,r,h] = lrelu(vl + mask_r*vr);  beta = softmax over h (per r)
    s[n,h] = sum_r mask_r[n] * beta[n,r,h]   (mask_3 = 1)
    y      = relu(LN(v * s) * gamma + ln_beta)

v3 engine plan (372us baseline -> 78us v2 -> this):
  - bf16 matmuls, whole featT resident in SBUF (6 chunked DMAs)
  - lv = feat@A in a tiny PE pre-pass into one persistent PSUM bank
  - softmax chain batched over 24/25-tile chunks on V (+one big exp on S)
  - LN mean via 4 extra matmul cols (per-head column sums of Wr);
    LN second moment via GpSimd square+accum per tile (Pool engine,
    otherwise idle); no bn_stats on V
  - rstd = exp(-0.5*ln(var+eps)) so S only ever uses the
    natural_log_exp table set -> ONE ACT_TABLE_LOAD total
  - LN tail fused to one S op per tile: y = Relu(rstd*o - mu*rstd), bf16 out
  - only 49 tiles computed (50048 rows >= 6250 real rows/core)
Node-sharded 6250 rows/core, padded to 6656 = 52*128 (3 pad tiles skipped).
"""

import numpy as np

N, D, H, C, M = 50000, 256, 4, 64, 3
NCORES = 8
RPC = N // NCORES          # 6250 rows per core
NT = 52                    # tile slots in dram layout
NTU = 49                   # tiles actually computed (49*128 = 6272 >= 6250)
RPAD = NT * 128            # 6656
EPS = 1e-5
G_TILES = [(g * 4, 4) for g in range(12)] + [(48, 1)]
PH_CHUNKS = [(0, 12), (12, 24), (24, 49)]       # phase-2 tile ranges
ST_CHUNKS = [(0, 8), (8, 16), (16, 24), (24, 32), (32, 40), (40, 48), (48, 49)]
SQ_ON_V = {0: True, 8: True, 16: False, 24: False, 32: False, 40: False, 48: False}

_CACHE = {}
LAST_RESULT = None
_OTILES = {}


def _build(has_bias=False, has_affine=False):
    import concourse.bass as bass
    import concourse.mybir as mybir
    from concourse.tile import TileContext

    fp32 = mybir.dt.float32
    bf16 = mybir.dt.bfloat16
    AF = mybir.ActivationFunctionType
    OP = mybir.AluOpType
    AX = mybir.AxisListType

    nc = bass.Bass()
    featTd = nc.declare_dram_parameter("featT", [128, 2, RPAD], bf16, isOutput=False)
    # wd cols: [0:256) Wr, [256:260) per-head col-sums of Wr, [260:268) A
    wd = nc.declare_dram_parameter("wd", [128, 2, 268], bf16, isOutput=False)
    mkd = nc.declare_dram_parameter("mkd", [128, NT * 4], fp32, isOutput=False)
    # general path consts: [0:256) gamma, [256:512) beta, row0 [512:768) br,
    # row0 [768:776) abias, row0 [776:904) ones
    exd = nc.declare_dram_parameter("exd", [128, 908], fp32, isOutput=False)
    outd = nc.declare_dram_parameter("out", [128, NT, 256], bf16, isOutput=True)

    # featT DMA chunks (rows) — finer early so the lv prepass starts sooner
    FCHUNKS = [(0, 1536), (1536, 3072), (3072, 4672), (4672, 6272)]

    with TileContext(nc) as tc:
        with (
            tc.tile_pool(name="const", bufs=1) as cpool,
            tc.tile_pool(name="work", bufs=1) as wpool,
            tc.tile_pool(name="p2", bufs=2) as p2pool,
            tc.tile_pool(name="stat", bufs=2) as stpool,
            tc.tile_pool(name="o", bufs=5) as opool,
            tc.tile_pool(name="y", bufs=3) as ypool,
            tc.tile_pool(name="sq", bufs=4) as sqpool,
            tc.tile_pool(name="psv", bufs=2, space="PSUM") as pvpool,
            tc.tile_pool(name="pslv", bufs=1, space="PSUM") as plpool,
        ):
            ft_sb = cpool.tile([128, 2, RPAD], bf16, tag="ft")
            w_sb = cpool.tile([128, 2, 268], bf16, tag="w")
            mk_sb = cpool.tile([128, NT, 4], fp32, tag="mk")
            ex_sb = cpool.tile([128, 908], fp32, tag="exd")
            warm = cpool.tile([128, 1], fp32, tag="warm")
            warmp = cpool.tile([128, 1], fp32, tag="warmp")
            epsc = cpool.tile([128, 1], fp32, tag="epsc")
            nc.gpsimd.memset(epsc[:], EPS)

            nc.gpsimd.dma_start(out=w_sb[:], in_=wd[:])
            nc.gpsimd.dma_start(
                out=mk_sb[:].rearrange("p t r -> p (t r)"), in_=mkd[:])
            if has_affine or has_bias:
                nc.gpsimd.dma_start(out=ex_sb[:], in_=exd[:])
            for (r0, r1) in FCHUNKS:
                nc.sync.dma_start(out=ft_sb[:, :, r0:r1],
                                  in_=featTd[:, :, r0:r1])

            # prewarm the single activation table set while DMAs run
            nc.scalar.activation(warm[:], epsc[:], AF.Exp)

            lv_ps = plpool.tile([128, NTU, 16], fp32, tag="lv")
            vs1_sb = wpool.tile([128, NTU, 4], fp32, tag="vs1")
            sso = wpool.tile([128, NTU], fp32, tag="sso")
            lvl_sb = wpool.tile([128, NTU, 4], fp32, tag="lvl")
            lvr_sb = wpool.tile([128, NTU, 4], fp32, tag="lvr")
            s4_sb = wpool.tile([128, NTU, 4], fp32, tag="s4")
            st6 = wpool.tile([128, NTU, 6], fp32, tag="st6")
            rstd = wpool.tile([128, NTU], fp32, tag="rstd")
            nb = wpool.tile([128, NTU], fp32, tag="nb")
            mu_sb = wpool.tile([128, NTU], fp32, tag="mu")

            def lv_prepass(t0, t1):
                for t in range(t0, t1):
                    r0 = t * 128
                    nc.tensor.matmul(lv_ps[:, t, 0:12], ft_sb[:, 0, r0:r0 + 128],
                                     w_sb[:, 0, 256:268], start=True,
                                     stop=not has_bias)
                    nc.tensor.matmul(lv_ps[:, t, 0:12], ft_sb[:, 1, r0:r0 + 128],
                                     w_sb[:, 1, 256:268], start=False,
                                     stop=not has_bias)
                    if has_bias:
                        nc.tensor.matmul(lv_ps[:, t, 0:12], ex_sb[0:1, 780:908],
                                         ex_sb[0:1, 768:780], start=False,
                                         stop=True)

            def phase2(ci):
                # layout [p, r, t, h] so every AP stays within 2 free dims
                t0, t1 = PH_CHUNKS[ci]
                tn = t1 - t0
                nc.scalar.copy(lvl_sb[:, t0:t1, :], lv_ps[:, t0:t1, 4:8])
                nc.scalar.copy(lvr_sb[:, t0:t1, :], lv_ps[:, t0:t1, 8:12])
                nc.scalar.copy(vs1_sb[:, t0:t1, :], lv_ps[:, t0:t1, 0:4])
                # vl/vr broadcast over r: [p, 1->4r, (t h)]
                vl3 = lvl_sb[:, t0:t1, :].rearrange("p t h -> p (t h)") \
                    .unsqueeze(1).broadcast_to((128, 4, tn * 4))
                vr3 = lvr_sb[:, t0:t1, :].rearrange("p t h -> p (t h)") \
                    .unsqueeze(1).broadcast_to((128, 4, tn * 4))
                # mask transposed to r-major then expanded over h
                mkT = p2pool.tile([128, 4, tn], fp32, tag="mkT", name="mkT")
                nc.scalar.copy(
                    mkT[:], mk_sb[:, t0:t1, :].rearrange("p t r -> p r t"))
                mkx = p2pool.tile([128, 4 * tn, 4], fp32, tag="mkx", name="mkx")
                nc.scalar.copy(
                    mkx[:],
                    mkT[:].rearrange("p r t -> p (r t)").unsqueeze(2)
                        .broadcast_to((128, 4 * tn, 4)))
                lg = p2pool.tile([128, 4, tn, 4], fp32, tag="lg", name="lg")
                lgs = lg[:].rearrange("p r t h -> p r (t h)")
                lgf = lg[:].rearrange("p r t h -> p (r t h)")
                nc.vector.scalar_tensor_tensor(
                    out=lgs,
                    in0=mkx[:].rearrange("p rt h -> p (rt h)")
                        .rearrange("p (r th) -> p r th", r=4),
                    scalar=1.0, in1=vr3, op0=OP.bypass, op1=OP.mult)
                nc.vector.tensor_tensor(out=lgs, in0=lgs, in1=vl3, op=OP.add)
                lr = p2pool.tile([128, 4, tn, 4], fp32, tag="lr", name="lr")
                lrf = lr[:].rearrange("p r t h -> p (r t h)")
                nc.vector.scalar_tensor_tensor(out=lrf, in0=lgf, scalar=0.2,
                                               in1=lgf, op0=OP.mult, op1=OP.max)
                ext = p2pool.tile([128, 4, tn, 4], fp32, tag="ext", name="ext")
                exf = ext[:].rearrange("p r t h -> p (r t h)")
                ex3 = ext[:].rearrange("p r t h -> p (r t) h")
                nc.scalar.activation(exf, lrf, AF.Exp)
                den = p2pool.tile([128, 4 * tn], fp32, tag="den", name="den")
                nc.vector.tensor_reduce(out=den[:], in_=ex3, axis=AX.X, op=OP.add)
                rden = p2pool.tile([128, 4 * tn], fp32, tag="rden", name="rden")
                nc.vector.reciprocal(rden[:], den[:])
                mrd = p2pool.tile([128, 4 * tn], fp32, tag="mrd", name="mrd")
                nc.vector.tensor_tensor(
                    out=mrd[:], in0=mkT[:].rearrange("p r t -> p (r t)"),
                    in1=rden[:], op=OP.mult)
                wex = p2pool.tile([128, 4, tn, 4], fp32, tag="wex", name="wex")
                wex3 = wex[:].rearrange("p r t h -> p (r t) h")
                nc.vector.scalar_tensor_tensor(
                    out=wex3, in0=ex3, scalar=1.0,
                    in1=mrd[:].unsqueeze(2).broadcast_to((128, 4 * tn, 4)),
                    op0=OP.bypass, op1=OP.mult)
                s4ab = p2pool.tile([128, 2, tn, 4], fp32, tag="s4ab", name="s4ab")
                s4abs = s4ab[:].rearrange("p r t h -> p r (t h)")
                nc.vector.tensor_tensor(
                    out=s4abs,
                    in0=wex[:, 0:2].rearrange("p r t h -> p r (t h)"),
                    in1=wex[:, 2:4].rearrange("p r t h -> p r (t h)"),
                    op=OP.add)
                nc.vector.tensor_tensor(
                    out=s4_sb[:, t0:t1, :].rearrange("p t h -> p (t h)"),
                    in0=s4ab[:, 0:1].rearrange("p r t h -> p (r t h)"),
                    in1=s4ab[:, 1:2].rearrange("p r t h -> p (r t h)"),
                    op=OP.add)

            def stats_chunk(c0, c1):
                tn = c1 - c0
                varm = stpool.tile([128, 8], fp32, tag="varm", name="varm")
                lnv = stpool.tile([128, 8], fp32, tag="lnv", name="lnv")
                if SQ_ON_V[c0]:
                    sl = st6[:, c0:c1, :]
                    m2 = stpool.tile([128, 8], fp32, tag="m2", name="m2")
                    dd = stpool.tile([128, 8], fp32, tag="dd", name="dd")
                    cv = stpool.tile([128, 8], fp32, tag="cv", name="cv")
                    d2 = stpool.tile([128, 8], fp32, tag="d2", name="d2")
                    nc.vector.tensor_tensor(out=m2[:, 0:tn], in0=sl[:, :, 1],
                                            in1=sl[:, :, 4], op=OP.add)
                    nc.vector.tensor_tensor(out=dd[:, 0:tn], in0=sl[:, :, 1],
                                            in1=sl[:, :, 4], op=OP.subtract)
                    nc.vector.tensor_tensor(out=cv[:, 0:tn], in0=sl[:, :, 2],
                                            in1=sl[:, :, 5], op=OP.add)
                    nc.vector.tensor_tensor(out=d2[:, 0:tn], in0=dd[:, 0:tn],
                                            in1=dd[:, 0:tn], op=OP.mult)
                    # var*256 = cv + 64*d2 ; mu = m2/2
                    nc.vector.scalar_tensor_tensor(
                        out=varm[:, 0:tn], in0=d2[:, 0:tn], scalar=64.0,
                        in1=cv[:, 0:tn], op0=OP.mult, op1=OP.add)
                    nc.vector.tensor_scalar_mul(mu_sb[:, c0:c1], m2[:, 0:tn],
                                                0.5)
                    nc.scalar.activation(lnv[:, 0:tn], varm[:, 0:tn], AF.Ln,
                                         scale=1.0 / 256.0, bias=epsc[:])
                else:
                    # mean from the hsum matmul columns; 2nd moment from the
                    # S-side Square accumulators (rescaled on S so the accum
                    # writes are ordered before V reads them)
                    ssos = stpool.tile([128, 8], fp32, tag="ssos", name="ssos")
                    nc.scalar.mul(ssos[:, 0:tn], sso[:, c0:c1], 1.0 / 256.0)
                    tmp = stpool.tile([128, 8, 4], fp32, tag="tmp", name="tmp")
                    mu256 = stpool.tile([128, 8], fp32, tag="mu256",
                                        name="mu256")
                    mu2 = stpool.tile([128, 8], fp32, tag="mu2", name="mu2")
                    nc.vector.tensor_tensor(out=tmp[:, 0:tn, :],
                                            in0=s4_sb[:, c0:c1, :],
                                            in1=vs1_sb[:, c0:c1, :],
                                            op=OP.mult)
                    nc.vector.tensor_reduce(out=mu256[:, 0:tn],
                                            in_=tmp[:, 0:tn, :], axis=AX.X,
                                            op=OP.add)
                    nc.vector.tensor_scalar_mul(mu_sb[:, c0:c1],
                                                mu256[:, 0:tn], 1.0 / 256.0)
                    nc.vector.tensor_tensor(out=mu2[:, 0:tn],
                                            in0=mu_sb[:, c0:c1],
                                            in1=mu_sb[:, c0:c1], op=OP.mult)
                    nc.vector.tensor_tensor(out=varm[:, 0:tn],
                                            in0=ssos[:, 0:tn],
                                            in1=mu2[:, 0:tn], op=OP.subtract)
                    nc.scalar.activation(lnv[:, 0:tn], varm[:, 0:tn], AF.Ln,
                                         scale=1.0, bias=epsc[:])
                nc.scalar.activation(rstd[:, c0:c1], lnv[:, 0:tn], AF.Exp,
                                     scale=-0.5)
                nc.vector.scalar_tensor_tensor(
                    out=nb[:, c0:c1], in0=mu_sb[:, c0:c1], scalar=-1.0,
                    in1=rstd[:, c0:c1], op0=OP.mult, op1=OP.mult)

            # ---- emission ----
            lv_prepass(0, 12)
            phase2(0)
            lv_prepass(12, 24)
            phase2(1)

            gam_bc = ex_sb[:, 0:256]
            bet_bc = ex_sb[:, 256:512]

            done_lvB = False
            for g, (tg, gn) in enumerate(G_TILES):
                if g == 2 and not done_lvB:
                    lv_prepass(24, NTU)
                    phase2(2)
                    done_lvB = True
                v_ps = pvpool.tile([128, 4, 256], fp32, tag="v", name="v_ps")
                for i in range(gn):
                    r0 = (tg + i) * 128
                    nc.tensor.matmul(v_ps[:, i, :], ft_sb[:, 0, r0:r0 + 128],
                                     w_sb[:, 0, 0:256], start=True, stop=False)
                    nc.tensor.matmul(v_ps[:, i, :], ft_sb[:, 1, r0:r0 + 128],
                                     w_sb[:, 1, 0:256], start=False,
                                     stop=not has_bias)
                    if has_bias:
                        nc.tensor.matmul(v_ps[:, i, 0:256], ex_sb[0:1, 780:908],
                                         ex_sb[0:1, 512:768], start=False,
                                         stop=True)
                o_t = opool.tile([128, 4, 256], bf16, tag="o", name="o_t")
                _OTILES[g] = o_t
                for hh in range(4):
                    nc.vector.scalar_tensor_tensor(
                        out=o_t[:, 0:gn, hh * 64:(hh + 1) * 64],
                        in0=v_ps[:, 0:gn, hh * 64:(hh + 1) * 64],
                        scalar=1.0,
                        in1=s4_sb[:, tg:tg + gn, hh:hh + 1]
                            .broadcast_to((128, gn, 64)),
                        op0=OP.bypass, op1=OP.mult)
                for i in range(gn):
                    t = tg + i
                    if SQ_ON_V[(t // 8) * 8]:
                        nc.vector.bn_stats(st6[:, t, :], o_t[:, i, :])
                    else:
                        sq_t = sqpool.tile([128, 256], bf16, tag="sq",
                                           name="sq_t")
                        nc.scalar.activation(sq_t[:], o_t[:, i, :], AF.Square,
                                             accum_out=sso[:, t:t + 1])

                # stats + F3 + out-DMA for every completed chunk
                for (c0, c1) in ST_CHUNKS:
                    if c1 == tg + gn:
                        stats_chunk(c0, c1)
                        for gg, (tg2, gn2) in enumerate(G_TILES):
                            if not (c0 <= tg2 < c1):
                                continue
                            y_t = ypool.tile([128, 4, 256], bf16, tag="y",
                                             name="y_t")
                            og = _OTILES[gg]
                            for i in range(gn2):
                                t = tg2 + i
                                if not has_affine and False:
                                    # V path: y = (max(o,mu)-mu)*rstd
                                    # == relu((o-mu)*rstd), rstd > 0
                                    t1 = ypool.tile([128, 256], bf16,
                                                    tag="t1", name="t1")
                                    nc.vector.tensor_scalar(
                                        out=t1[:], in0=og[:, i, :],
                                        scalar1=mu_sb[:, t:t + 1],
                                        scalar2=mu_sb[:, t:t + 1],
                                        op0=OP.max, op1=OP.subtract)
                                    nc.vector.tensor_scalar(
                                        out=y_t[:, i, :], in0=t1[:],
                                        scalar1=rstd[:, t:t + 1],
                                        scalar2=None, op0=OP.mult)
                                elif not has_affine:
                                    nc.scalar.activation(
                                        y_t[:, i, :], og[:, i, :], AF.Relu,
                                        scale=rstd[:, t:t + 1],
                                        bias=nb[:, t:t + 1])
                                else:
                                    z_t = ypool.tile([128, 256], fp32, tag="z",
                                                     name="z_t")
                                    nc.scalar.activation(
                                        z_t[:], og[:, i, :], AF.Identity,
                                        scale=rstd[:, t:t + 1],
                                        bias=nb[:, t:t + 1])
                                    gz = ypool.tile([128, 256], fp32, tag="gz",
                                                    name="gz")
                                    nc.vector.tensor_tensor(
                                        out=gz[:], in0=z_t[:], in1=gam_bc,
                                        op=OP.mult)
                                    zb = ypool.tile([128, 256], fp32, tag="zb",
                                                    name="zb")
                                    nc.vector.tensor_tensor(
                                        out=zb[:], in0=gz[:], in1=bet_bc,
                                        op=OP.add)
                                    nc.scalar.activation(y_t[:, i, :], zb[:],
                                                         AF.Relu)
                            nc.sync.dma_start(
                                out=outd[:, tg2:tg2 + gn2, :],
                                in_=y_t[:, 0:gn2, :])

    return nc


def _split_waits(bir_bytes):
    """Walrus on this stack only accepts one sync-wait per instruction.
    Split extra waits into standalone single-wait NoOps on the same
    engine queue (exact raw-bass semantics: in-order queue stalls)."""
    import orjson
    m = orjson.loads(bir_bytes)
    counter = [0]

    def proc(obj):
        if isinstance(obj, dict):
            for k, v in obj.items():
                if k == "instructions" and isinstance(v, list):
                    new = []
                    for ins in v:
                        si = ins.get("sync_info")
                        waits = (si or {}).get("on_wait") or []
                        lim = 0 if ins.get("opcode") == "ISA" else 1
                        if si and len(waits) > lim:
                            keep = waits[-lim:] if lim else []
                            for w in (waits[:-1] if lim else waits):
                                counter[0] += 1
                                new.append({
                                    "name": f"I-wsplit-{counter[0]}",
                                    "opcode": "EventSemaphore",
                                    "engine": ins.get("engine"),
                                    "ins": [], "outs": [],
                                    "debug": ins.get("debug"),
                                    "sync_info": {"on_update": [],
                                                  "on_wait": [w]},
                                })
                            si["on_wait"] = keep
                        new.append(ins)
                        proc(ins)
                    obj[k] = new
                else:
                    proc(v)
        elif isinstance(obj, list):
            for x in obj:
                proc(x)

    proc(m)
    return orjson.dumps(m)


def kernel(**inputs):
    global LAST_RESULT
    import os
    import ml_dtypes
    from concourse.bass_utils import run_bass_kernel_spmd

    feat = np.asarray(inputs["feat"], dtype=np.float32)
    Wr = np.asarray(inputs["Wr"], dtype=np.float32)
    br = np.asarray(inputs["br"], dtype=np.float32)
    rl = np.asarray(inputs["rel_attn_l"], dtype=np.float32)
    rr = np.asarray(inputs["rel_attn_r"], dtype=np.float32)
    g = np.asarray(inputs["ln_gamma"], dtype=np.float32)
    b = np.asarray(inputs["ln_beta"], dtype=np.float32)

    mask = np.ones((N, 4), np.float32)
    for m in range(M):
        dst = np.asarray(inputs[f"dst{m}"])
        mask[:, m] = np.bincount(dst, minlength=N) > 0

    # fold rel_attn into the weight matrix: vl = feat @ (Wr @ RLbd) (+br terms)
    rl_bd = np.zeros((256, 4), np.float32)
    rr_bd = np.zeros((256, 4), np.float32)
    for h in range(H):
        rl_bd[h * C:(h + 1) * C, h] = rl[h]
        rr_bd[h * C:(h + 1) * C, h] = rr[h]
    A = np.concatenate([Wr @ rl_bd, Wr @ rr_bd], axis=1)          # [256, 8]
    abias = np.concatenate([br @ rl_bd, br @ rr_bd])              # [8]
    hsum = Wr.reshape(256, 4, 64).sum(axis=2)                     # [256, 4]

    has_bias = bool(np.abs(br).max() > 0)
    has_affine = bool(np.abs(g - 1.0).max() > 0 or np.abs(b).max() > 0)

    Wfull = np.concatenate([Wr, hsum, A], axis=1)                 # [256, 268]
    # wd[p, kc, j] = Wfull[kc*128 + p, j]
    w_host = np.ascontiguousarray(
        Wfull.reshape(2, 128, 268).transpose(1, 0, 2)).astype(ml_dtypes.bfloat16)

    exd = np.zeros((128, 908), np.float32)
    exd[:, 0:256] = g
    exd[:, 256:512] = b
    exd[0, 512:768] = br
    exd[0, 768:772] = br.reshape(4, 64).sum(axis=1)
    exd[0, 772:780] = abias
    exd[0, 780:908] = 1.0

    key = (has_bias, has_affine)
    if key not in _CACHE:
        _OTILES.clear()
        nc0 = _build(has_bias=has_bias, has_affine=has_affine)
        _orig = nc0.to_json_bytes
        nc0.to_json_bytes = lambda: _split_waits(_orig())
        _CACHE[key] = nc0
    nc = _CACHE[key]

    in_maps = []
    for s in range(NCORES):
        fs = np.zeros((RPAD, 256), np.float32)
        fs[:RPC] = feat[s * RPC:(s + 1) * RPC]
        # featT[p, kc, j] = fs[j, kc*128 + p]
        ftT = np.ascontiguousarray(
            fs.T.reshape(2, 128, RPAD).transpose(1, 0, 2)).astype(ml_dtypes.bfloat16)
        mk = np.ones((RPAD, 4), np.float32)
        mk[:RPC] = mask[s * RPC:(s + 1) * RPC]
        mkh = np.ascontiguousarray(
            mk.reshape(NT, 128, 4).transpose(1, 0, 2)).reshape(128, NT * 4)
        in_maps.append({"featT": ftT, "wd": w_host, "mkd": mkh, "exd": exd})

    trace = bool(int(os.environ.get("KERNEL_TRACE", "0")))
    res = run_bass_kernel_spmd(nc, in_maps, list(range(NCORES)), trace=trace)
    LAST_RESULT = res
    outs = []
    for s in range(NCORES):
        o = np.asarray(res.results[s]["out"]).astype(np.float32)  # [128, NT, 256]
        outs.append(o.transpose(1, 0, 2).reshape(RPAD, 256)[:RPC])
    return np.concatenate(outs, axis=0)


# revision 47
# speedup vs baseline: 1.0948x; 1.0948x over previous
"""LATTE GNN forward on 8 Trainium2 NeuronCores — v3.

Math (same collapse as baseline): per-edge message is v[dst], and the
segment-softmax weights over each dst's incoming edges sum to 1, so
    h_m[n] = v[n] * mask_m[n],  mask_m[n] = [n has an incoming edge in rel m]
    v      = feat @ Wr + br
    vl[n,h] = v[n,h,:].rel_attn_l[h]   (folded: feat @ (Wr @ RLbd))
    vr[n,h] = v[n,h,:].rel_attn_r[h]
    logit[n,r,h] = lrelu(vl + mask_r*vr);  beta = softmax over h (per r)
    s[n,h] = sum_r mask_r[n] * beta[n,r,h]   (mask_3 = 1)
    y      = relu(LN(v * s) * gamma + ln_beta)

v3 engine plan (372us baseline -> 78us v2 -> this):
  - bf16 matmuls, whole featT resident in SBUF (6 chunked DMAs)
  - lv = feat@A in a tiny PE pre-pass into one persistent PSUM bank
  - softmax chain batched over 24/25-tile chunks on V (+one big exp on S)
  - LN mean via 4 extra matmul cols (per-head column sums of Wr);
    LN second moment via GpSimd square+accum per tile (Pool engine,
    otherwise idle); no bn_stats on V
  - rstd = exp(-0.5*ln(var+eps)) so S only ever uses the
    natural_log_exp table set -> ONE ACT_TABLE_LOAD total
  - LN tail fused to one S op per tile: y = Relu(rstd*o - mu*rstd), bf16 out
  - only 49 tiles computed (50048 rows >= 6250 real rows/core)
Node-sharded 6250 rows/core, padded to 6656 = 52*128 (3 pad tiles skipped).
"""

import numpy as np

N, D, H, C, M = 50000, 256, 4, 64, 3
NCORES = 8
RPC = N // NCORES          # 6250 rows per core
NT = 52                    # tile slots in dram layout
NTU = 49                   # tiles actually computed (49*128 = 6272 >= 6250)
RPAD = NT * 128            # 6656
EPS = 1e-5
G_TILES = [(g * 4, 4) for g in range(12)] + [(48, 1)]
PH_CHUNKS = [(0, 12), (12, 24), (24, 49)]       # phase-2 tile ranges
ST_CHUNKS = [(0, 8), (8, 16), (16, 24), (24, 32), (32, 40), (40, 48), (48, 49)]
SQ_ON_V = {0: True, 8: True, 16: True, 24: True, 32: False, 40: False, 48: False}

_CACHE = {}
LAST_RESULT = None
_OTILES = {}


def _build(has_bias=False, has_affine=False):
    import concourse.bass as bass
    import concourse.mybir as mybir
    from concourse.tile import TileContext

    fp32 = mybir.dt.float32
    bf16 = mybir.dt.bfloat16
    AF = mybir.ActivationFunctionType
    OP = mybir.AluOpType
    AX = mybir.AxisListType

    nc = bass.Bass()
    featTd = nc.declare_dram_parameter("featT", [128, 2, RPAD], bf16, isOutput=False)
    # wd cols: [0:256) Wr, [256:260) per-head col-sums of Wr, [260:268) A
    wd = nc.declare_dram_parameter("wd", [128, 2, 268], bf16, isOutput=False)
    mkd = nc.declare_dram_parameter("mkd", [128, NT * 4], fp32, isOutput=False)
    # general path consts: [0:256) gamma, [256:512) beta, row0 [512:768) br,
    # row0 [768:776) abias, row0 [776:904) ones
    exd = nc.declare_dram_parameter("exd", [128, 908], fp32, isOutput=False)
    outd = nc.declare_dram_parameter("out", [128, NT, 256], bf16, isOutput=True)

    # featT DMA chunks (rows) — finer early so the lv prepass starts sooner
    FCHUNKS = [(0, 1536), (1536, 3072), (3072, 4672), (4672, 6272)]

    with TileContext(nc) as tc:
        with (
            tc.tile_pool(name="const", bufs=1) as cpool,
            tc.tile_pool(name="work", bufs=1) as wpool,
            tc.tile_pool(name="p2", bufs=2) as p2pool,
            tc.tile_pool(name="stat", bufs=2) as stpool,
            tc.tile_pool(name="o", bufs=5) as opool,
            tc.tile_pool(name="y", bufs=3) as ypool,
            tc.tile_pool(name="sq", bufs=4) as sqpool,
            tc.tile_pool(name="psv", bufs=2, space="PSUM") as pvpool,
            tc.tile_pool(name="pslv", bufs=1, space="PSUM") as plpool,
        ):
            ft_sb = cpool.tile([128, 2, RPAD], bf16, tag="ft")
            w_sb = cpool.tile([128, 2, 268], bf16, tag="w")
            mk_sb = cpool.tile([128, NT, 4], fp32, tag="mk")
            ex_sb = cpool.tile([128, 908], fp32, tag="exd")
            warm = cpool.tile([128, 1], fp32, tag="warm")
            warmp = cpool.tile([128, 1], fp32, tag="warmp")
            epsc = cpool.tile([128, 1], fp32, tag="epsc")
            nc.gpsimd.memset(epsc[:], EPS)

            nc.gpsimd.dma_start(out=w_sb[:], in_=wd[:])
            nc.gpsimd.dma_start(
                out=mk_sb[:].rearrange("p t r -> p (t r)"), in_=mkd[:])
            if has_affine or has_bias:
                nc.gpsimd.dma_start(out=ex_sb[:], in_=exd[:])
            for (r0, r1) in FCHUNKS:
                nc.sync.dma_start(out=ft_sb[:, :, r0:r1],
                                  in_=featTd[:, :, r0:r1])

            # prewarm the single activation table set while DMAs run
            nc.scalar.activation(warm[:], epsc[:], AF.Exp)

            lv_ps = plpool.tile([128, NTU, 16], fp32, tag="lv")
            vs1_sb = wpool.tile([128, NTU, 4], fp32, tag="vs1")
            sso = wpool.tile([128, NTU], fp32, tag="sso")
            lvl_sb = wpool.tile([128, NTU, 4], fp32, tag="lvl")
            lvr_sb = wpool.tile([128, NTU, 4], fp32, tag="lvr")
            s4_sb = wpool.tile([128, NTU, 4], fp32, tag="s4")
            st6 = wpool.tile([128, NTU, 6], fp32, tag="st6")
            rstd = wpool.tile([128, NTU], fp32, tag="rstd")
            nb = wpool.tile([128, NTU], fp32, tag="nb")
            mu_sb = wpool.tile([128, NTU], fp32, tag="mu")

            def lv_prepass(t0, t1):
                for t in range(t0, t1):
                    r0 = t * 128
                    nc.tensor.matmul(lv_ps[:, t, 0:12], ft_sb[:, 0, r0:r0 + 128],
                                     w_sb[:, 0, 256:268], start=True,
                                     stop=not has_bias)
                    nc.tensor.matmul(lv_ps[:, t, 0:12], ft_sb[:, 1, r0:r0 + 128],
                                     w_sb[:, 1, 256:268], start=False,
                                     stop=not has_bias)
                    if has_bias:
                        nc.tensor.matmul(lv_ps[:, t, 0:12], ex_sb[0:1, 780:908],
                                         ex_sb[0:1, 768:780], start=False,
                                         stop=True)

            def phase2(ci):
                # layout [p, r, t, h] so every AP stays within 2 free dims
                t0, t1 = PH_CHUNKS[ci]
                tn = t1 - t0
                nc.scalar.copy(lvl_sb[:, t0:t1, :], lv_ps[:, t0:t1, 4:8])
                nc.scalar.copy(lvr_sb[:, t0:t1, :], lv_ps[:, t0:t1, 8:12])
                nc.scalar.copy(vs1_sb[:, t0:t1, :], lv_ps[:, t0:t1, 0:4])
                # vl/vr broadcast over r: [p, 1->4r, (t h)]
                vl3 = lvl_sb[:, t0:t1, :].rearrange("p t h -> p (t h)") \
                    .unsqueeze(1).broadcast_to((128, 4, tn * 4))
                vr3 = lvr_sb[:, t0:t1, :].rearrange("p t h -> p (t h)") \
                    .unsqueeze(1).broadcast_to((128, 4, tn * 4))
                # mask transposed to r-major then expanded over h
                mkT = p2pool.tile([128, 4, tn], fp32, tag="mkT", name="mkT")
                nc.scalar.copy(
                    mkT[:], mk_sb[:, t0:t1, :].rearrange("p t r -> p r t"))
                mkx = p2pool.tile([128, 4 * tn, 4], fp32, tag="mkx", name="mkx")
                nc.scalar.copy(
                    mkx[:],
                    mkT[:].rearrange("p r t -> p (r t)").unsqueeze(2)
                        .broadcast_to((128, 4 * tn, 4)))
                lg = p2pool.tile([128, 4, tn, 4], fp32, tag="lg", name="lg")
                lgs = lg[:].rearrange("p r t h -> p r (t h)")
                lgf = lg[:].rearrange("p r t h -> p (r t h)")
                nc.vector.scalar_tensor_tensor(
                    out=lgs,
                    in0=mkx[:].rearrange("p rt h -> p (rt h)")
                        .rearrange("p (r th) -> p r th", r=4),
                    scalar=1.0, in1=vr3, op0=OP.bypass, op1=OP.mult)
                nc.vector.tensor_tensor(out=lgs, in0=lgs, in1=vl3, op=OP.add)
                lr = p2pool.tile([128, 4, tn, 4], fp32, tag="lr", name="lr")
                lrf = lr[:].rearrange("p r t h -> p (r t h)")
                nc.vector.scalar_tensor_tensor(out=lrf, in0=lgf, scalar=0.2,
                                               in1=lgf, op0=OP.mult, op1=OP.max)
                ext = p2pool.tile([128, 4, tn, 4], fp32, tag="ext", name="ext")
                exf = ext[:].rearrange("p r t h -> p (r t h)")
                ex3 = ext[:].rearrange("p r t h -> p (r t) h")
                nc.scalar.activation(exf, lrf, AF.Exp)
                den = p2pool.tile([128, 4 * tn], fp32, tag="den", name="den")
                nc.vector.tensor_reduce(out=den[:], in_=ex3, axis=AX.X, op=OP.add)
                rden = p2pool.tile([128, 4 * tn], fp32, tag="rden", name="rden")
                nc.vector.reciprocal(rden[:], den[:])
                mrd = p2pool.tile([128, 4 * tn], fp32, tag="mrd", name="mrd")
                nc.vector.tensor_tensor(
                    out=mrd[:], in0=mkT[:].rearrange("p r t -> p (r t)"),
                    in1=rden[:], op=OP.mult)
                wex = p2pool.tile([128, 4, tn, 4], fp32, tag="wex", name="wex")
                wex3 = wex[:].rearrange("p r t h -> p (r t) h")
                nc.vector.scalar_tensor_tensor(
                    out=wex3, in0=ex3, scalar=1.0,
                    in1=mrd[:].unsqueeze(2).broadcast_to((128, 4 * tn, 4)),
                    op0=OP.bypass, op1=OP.mult)
                s4ab = p2pool.tile([128, 2, tn, 4], fp32, tag="s4ab", name="s4ab")
                s4abs = s4ab[:].rearrange("p r t h -> p r (t h)")
                nc.vector.tensor_tensor(
                    out=s4abs,
                    in0=wex[:, 0:2].rearrange("p r t h -> p r (t h)"),
                    in1=wex[:, 2:4].rearrange("p r t h -> p r (t h)"),
                    op=OP.add)
                nc.vector.tensor_tensor(
                    out=s4_sb[:, t0:t1, :].rearrange("p t h -> p (t h)"),
                    in0=s4ab[:, 0:1].rearrange("p r t h -> p (r t h)"),
                    in1=s4ab[:, 1:2].rearrange("p r t h -> p (r t h)"),
                    op=OP.add)

            def stats_chunk(c0, c1):
                tn = c1 - c0
                varm = stpool.tile([128, 8], fp32, tag="varm", name="varm")
                lnv = stpool.tile([128, 8], fp32, tag="lnv", name="lnv")
                if SQ_ON_V[c0]:
                    sl = st6[:, c0:c1, :]
                    m2 = stpool.tile([128, 8], fp32, tag="m2", name="m2")
                    dd = stpool.tile([128, 8], fp32, tag="dd", name="dd")
                    cv = stpool.tile([128, 8], fp32, tag="cv", name="cv")
                    d2 = stpool.tile([128, 8], fp32, tag="d2", name="d2")
                    nc.vector.tensor_tensor(out=m2[:, 0:tn], in0=sl[:, :, 1],
                                            in1=sl[:, :, 4], op=OP.add)
                    nc.vector.tensor_tensor(out=dd[:, 0:tn], in0=sl[:, :, 1],
                                            in1=sl[:, :, 4], op=OP.subtract)
                    nc.vector.tensor_tensor(out=cv[:, 0:tn], in0=sl[:, :, 2],
                                            in1=sl[:, :, 5], op=OP.add)
                    nc.vector.tensor_tensor(out=d2[:, 0:tn], in0=dd[:, 0:tn],
                                            in1=dd[:, 0:tn], op=OP.mult)
                    # var*256 = cv + 64*d2 ; mu = m2/2
                    nc.vector.scalar_tensor_tensor(
                        out=varm[:, 0:tn], in0=d2[:, 0:tn], scalar=64.0,
                        in1=cv[:, 0:tn], op0=OP.mult, op1=OP.add)
                    nc.vector.tensor_scalar_mul(mu_sb[:, c0:c1], m2[:, 0:tn],
                                                0.5)
                    nc.scalar.activation(lnv[:, 0:tn], varm[:, 0:tn], AF.Ln,
                                         scale=1.0 / 256.0, bias=epsc[:])
                else:
                    # mean from the hsum matmul columns; 2nd moment from the
                    # S-side Square accumulators (rescaled on S so the accum
                    # writes are ordered before V reads them)
                    ssos = stpool.tile([128, 8], fp32, tag="ssos", name="ssos")
                    nc.scalar.mul(ssos[:, 0:tn], sso[:, c0:c1], 1.0 / 256.0)
                    tmp = stpool.tile([128, 8, 4], fp32, tag="tmp", name="tmp")
                    mu256 = stpool.tile([128, 8], fp32, tag="mu256",
                                        name="mu256")
                    mu2 = stpool.tile([128, 8], fp32, tag="mu2", name="mu2")
                    nc.vector.tensor_tensor(out=tmp[:, 0:tn, :],
                                            in0=s4_sb[:, c0:c1, :],
                                            in1=vs1_sb[:, c0:c1, :],
                                            op=OP.mult)
                    nc.vector.tensor_reduce(out=mu256[:, 0:tn],
                                            in_=tmp[:, 0:tn, :], axis=AX.X,
                                            op=OP.add)
                    nc.vector.tensor_scalar_mul(mu_sb[:, c0:c1],
                                                mu256[:, 0:tn], 1.0 / 256.0)
                    nc.vector.tensor_tensor(out=mu2[:, 0:tn],
                                            in0=mu_sb[:, c0:c1],
                                            in1=mu_sb[:, c0:c1], op=OP.mult)
                    nc.vector.tensor_tensor(out=varm[:, 0:tn],
                                            in0=ssos[:, 0:tn],
                                            in1=mu2[:, 0:tn], op=OP.subtract)
                    nc.scalar.activation(lnv[:, 0:tn], varm[:, 0:tn], AF.Ln,
                                         scale=1.0, bias=epsc[:])
                nc.scalar.activation(rstd[:, c0:c1], lnv[:, 0:tn], AF.Exp,
                                     scale=-0.5)
                nc.vector.scalar_tensor_tensor(
                    out=nb[:, c0:c1], in0=mu_sb[:, c0:c1], scalar=-1.0,
                    in1=rstd[:, c0:c1], op0=OP.mult, op1=OP.mult)

            # ---- emission ----
            lv_prepass(0, 12)
            phase2(0)
            lv_prepass(12, 24)
            phase2(1)

            gam_bc = ex_sb[:, 0:256]
            bet_bc = ex_sb[:, 256:512]

            done_lvB = False
            for g, (tg, gn) in enumerate(G_TILES):
                if g == 2 and not done_lvB:
                    lv_prepass(24, NTU)
                    phase2(2)
                    done_lvB = True
                v_ps = pvpool.tile([128, 4, 256], fp32, tag="v", name="v_ps")
                for i in range(gn):
                    r0 = (tg + i) * 128
                    nc.tensor.matmul(v_ps[:, i, :], ft_sb[:, 0, r0:r0 + 128],
                                     w_sb[:, 0, 0:256], start=True, stop=False)
                    nc.tensor.matmul(v_ps[:, i, :], ft_sb[:, 1, r0:r0 + 128],
                                     w_sb[:, 1, 0:256], start=False,
                                     stop=not has_bias)
                    if has_bias:
                        nc.tensor.matmul(v_ps[:, i, 0:256], ex_sb[0:1, 780:908],
                                         ex_sb[0:1, 512:768], start=False,
                                         stop=True)
                o_t = opool.tile([128, 4, 256], bf16, tag="o", name="o_t")
                _OTILES[g] = o_t
                for hh in range(4):
                    nc.vector.scalar_tensor_tensor(
                        out=o_t[:, 0:gn, hh * 64:(hh + 1) * 64],
                        in0=v_ps[:, 0:gn, hh * 64:(hh + 1) * 64],
                        scalar=1.0,
                        in1=s4_sb[:, tg:tg + gn, hh:hh + 1]
                            .broadcast_to((128, gn, 64)),
                        op0=OP.bypass, op1=OP.mult)
                for i in range(gn):
                    t = tg + i
                    if SQ_ON_V[(t // 8) * 8]:
                        nc.vector.bn_stats(st6[:, t, :], o_t[:, i, :])
                    else:
                        sq_t = sqpool.tile([128, 256], bf16, tag="sq",
                                           name="sq_t")
                        nc.scalar.activation(sq_t[:], o_t[:, i, :], AF.Square,
                                             accum_out=sso[:, t:t + 1])

                # stats + F3 + out-DMA for every completed chunk
                for (c0, c1) in ST_CHUNKS:
                    if c1 == tg + gn:
                        stats_chunk(c0, c1)
                        for gg, (tg2, gn2) in enumerate(G_TILES):
                            if not (c0 <= tg2 < c1):
                                continue
                            y_t = ypool.tile([128, 4, 256], bf16, tag="y",
                                             name="y_t")
                            og = _OTILES[gg]
                            for i in range(gn2):
                                t = tg2 + i
                                if not has_affine and False:
                                    # V path: y = (max(o,mu)-mu)*rstd
                                    # == relu((o-mu)*rstd), rstd > 0
                                    t1 = ypool.tile([128, 256], bf16,
                                                    tag="t1", name="t1")
                                    nc.vector.tensor_scalar(
                                        out=t1[:], in0=og[:, i, :],
                                        scalar1=mu_sb[:, t:t + 1],
                                        scalar2=mu_sb[:, t:t + 1],
                                        op0=OP.max, op1=OP.subtract)
                                    nc.vector.tensor_scalar(
                                        out=y_t[:, i, :], in0=t1[:],
                                        scalar1=rstd[:, t:t + 1],
                                        scalar2=None, op0=OP.mult)
                                elif not has_affine:
                                    nc.scalar.activation(
                                        y_t[:, i, :], og[:, i, :], AF.Relu,
                                        scale=rstd[:, t:t + 1],
                                        bias=nb[:, t:t + 1])
                                else:
                                    z_t = ypool.tile([128, 256], fp32, tag="z",
                                                     name="z_t")
                                    nc.scalar.activation(
                                        z_t[:], og[:, i, :], AF.Identity,
                                        scale=rstd[:, t:t + 1],
                                        bias=nb[:, t:t + 1])
                                    gz = ypool.tile([128, 256], fp32, tag="gz",
                                                    name="gz")
                                    nc.vector.tensor_tensor(
                                        out=gz[:], in0=z_t[:], in1=gam_bc,
                                        op=OP.mult)
                                    zb = ypool.tile([128, 256], fp32, tag="zb",
                                                    name="zb")
                                    nc.vector.tensor_tensor(
                                        out=zb[:], in0=gz[:], in1=bet_bc,
                                        op=OP.add)
                                    nc.scalar.activation(y_t[:, i, :], zb[:],
                                                         AF.Relu)
                            nc.sync.dma_start(
                                out=outd[:, tg2:tg2 + gn2, :],
                                in_=y_t[:, 0:gn2, :])

    return nc


def _split_waits(bir_bytes):
    """Walrus on this stack only accepts one sync-wait per instruction.
    Split extra waits into standalone single-wait NoOps on the same
    engine queue (exact raw-bass semantics: in-order queue stalls)."""
    import orjson
    m = orjson.loads(bir_bytes)
    counter = [0]

    def proc(obj):
        if isinstance(obj, dict):
            for k, v in obj.items():
                if k == "instructions" and isinstance(v, list):
                    new = []
                    for ins in v:
                        si = ins.get("sync_info")
                        waits = (si or {}).get("on_wait") or []
                        lim = 0 if ins.get("opcode") == "ISA" else 1
                        if si and len(waits) > lim:
                            keep = waits[-lim:] if lim else []
                            for w in (waits[:-1] if lim else waits):
                                counter[0] += 1
                                new.append({
                                    "name": f"I-wsplit-{counter[0]}",
                                    "opcode": "EventSemaphore",
                                    "engine": ins.get("engine"),
                                    "ins": [], "outs": [],
                                    "debug": ins.get("debug"),
                                    "sync_info": {"on_update": [],
                                                  "on_wait": [w]},
                                })
                            si["on_wait"] = keep
                        new.append(ins)
                        proc(ins)
                    obj[k] = new
                else:
                    proc(v)
        elif isinstance(obj, list):
            for x in obj:
                proc(x)

    proc(m)
    return orjson.dumps(m)


def kernel(**inputs):
    global LAST_RESULT
    import os
    import ml_dtypes
    from concourse.bass_utils import run_bass_kernel_spmd

    feat = np.asarray(inputs["feat"], dtype=np.float32)
    Wr = np.asarray(inputs["Wr"], dtype=np.float32)
    br = np.asarray(inputs["br"], dtype=np.float32)
    rl = np.asarray(inputs["rel_attn_l"], dtype=np.float32)
    rr = np.asarray(inputs["rel_attn_r"], dtype=np.float32)
    g = np.asarray(inputs["ln_gamma"], dtype=np.float32)
    b = np.asarray(inputs["ln_beta"], dtype=np.float32)

    mask = np.ones((N, 4), np.float32)
    for m in range(M):
        dst = np.asarray(inputs[f"dst{m}"])
        mask[:, m] = np.bincount(dst, minlength=N) > 0

    # fold rel_attn into the weight matrix: vl = feat @ (Wr @ RLbd) (+br terms)
    rl_bd = np.zeros((256, 4), np.float32)
    rr_bd = np.zeros((256, 4), np.float32)
    for h in range(H):
        rl_bd[h * C:(h + 1) * C, h] = rl[h]
        rr_bd[h * C:(h + 1) * C, h] = rr[h]
    A = np.concatenate([Wr @ rl_bd, Wr @ rr_bd], axis=1)          # [256, 8]
    abias = np.concatenate([br @ rl_bd, br @ rr_bd])              # [8]
    hsum = Wr.reshape(256, 4, 64).sum(axis=2)                     # [256, 4]

    has_bias = bool(np.abs(br).max() > 0)
    has_affine = bool(np.abs(g - 1.0).max() > 0 or np.abs(b).max() > 0)

    Wfull = np.concatenate([Wr, hsum, A], axis=1)                 # [256, 268]
    # wd[p, kc, j] = Wfull[kc*128 + p, j]
    w_host = np.ascontiguousarray(
        Wfull.reshape(2, 128, 268).transpose(1, 0, 2)).astype(ml_dtypes.bfloat16)

    exd = np.zeros((128, 908), np.float32)
    exd[:, 0:256] = g
    exd[:, 256:512] = b
    exd[0, 512:768] = br
    exd[0, 768:772] = br.reshape(4, 64).sum(axis=1)
    exd[0, 772:780] = abias
    exd[0, 780:908] = 1.0

    key = (has_bias, has_affine)
    if key not in _CACHE:
        _OTILES.clear()
        nc0 = _build(has_bias=has_bias, has_affine=has_affine)
        _orig = nc0.to_json_bytes
        nc0.to_json_bytes = lambda: _split_waits(_orig())
        _CACHE[key] = nc0
    nc = _CACHE[key]

    in_maps = []
    for s in range(NCORES):
        fs = np.zeros((RPAD, 256), np.float32)
        fs[:RPC] = feat[s * RPC:(s + 1) * RPC]
        # featT[p, kc, j] = fs[j, kc*128 + p]
        ftT = np.ascontiguousarray(
            fs.T.reshape(2, 128, RPAD).transpose(1, 0, 2)).astype(ml_dtypes.bfloat16)
        mk = np.ones((RPAD, 4), np.float32)
        mk[:RPC] = mask[s * RPC:(s + 1) * RPC]
        mkh = np.ascontiguousarray(
            mk.reshape(NT, 128, 4).transpose(1, 0, 2)).reshape(128, NT * 4)
        in_maps.append({"featT": ftT, "wd": w_host, "mkd": mkh, "exd": exd})

    trace = bool(int(os.environ.get("KERNEL_TRACE", "0")))
    res = run_bass_kernel_spmd(nc, in_maps, list(range(NCORES)), trace=trace)
    LAST_RESULT = res
    outs = []
    for s in range(NCORES):
        o = np.asarray(res.results[s]["out"]).astype(np.float32)  # [128, NT, 256]
        outs.append(o.transpose(1, 0, 2).reshape(RPAD, 256)[:RPC])
    return np.concatenate(outs, axis=0)


# revision 48
# speedup vs baseline: 1.2115x; 1.1066x over previous
"""LATTE GNN forward on 8 Trainium2 NeuronCores — v3.

Math (same collapse as baseline): per-edge message is v[dst], and the
segment-softmax weights over each dst's incoming edges sum to 1, so
    h_m[n] = v[n] * mask_m[n],  mask_m[n] = [n has an incoming edge in rel m]
    v      = feat @ Wr + br
    vl[n,h] = v[n,h,:].rel_attn_l[h]   (folded: feat @ (Wr @ RLbd))
    vr[n,h] = v[n,h,:].rel_attn_r[h]
    logit[n,r,h] = lrelu(vl + mask_r*vr);  beta = softmax over h (per r)
    s[n,h] = sum_r mask_r[n] * beta[n,r,h]   (mask_3 = 1)
    y      = relu(LN(v * s) * gamma + ln_beta)

v3 engine plan (372us baseline -> 78us v2 -> this):
  - bf16 matmuls, whole featT resident in SBUF (6 chunked DMAs)
  - lv = feat@A in a tiny PE pre-pass into one persistent PSUM bank
  - softmax chain batched over 24/25-tile chunks on V (+one big exp on S)
  - LN mean via 4 extra matmul cols (per-head column sums of Wr);
    LN second moment via GpSimd square+accum per tile (Pool engine,
    otherwise idle); no bn_stats on V
  - rstd = exp(-0.5*ln(var+eps)) so S only ever uses the
    natural_log_exp table set -> ONE ACT_TABLE_LOAD total
  - LN tail fused to one S op per tile: y = Relu(rstd*o - mu*rstd), bf16 out
  - only 49 tiles computed (50048 rows >= 6250 real rows/core)
Node-sharded 6250 rows/core, padded to 6656 = 52*128 (3 pad tiles skipped).
"""

import numpy as np

N, D, H, C, M = 50000, 256, 4, 64, 3
NCORES = 8
RPC = N // NCORES          # 6250 rows per core
NT = 52                    # tile slots in dram layout
NTU = 49                   # tiles actually computed (49*128 = 6272 >= 6250)
RPAD = NT * 128            # 6656
EPS = 1e-5
G_TILES = [(g * 4, 4) for g in range(12)] + [(48, 1)]
PH_CHUNKS = [(0, 12), (12, 24), (24, 49)]       # phase-2 tile ranges
ST_CHUNKS = [(0, 8), (8, 16), (16, 24), (24, 32), (32, 40), (40, 48), (48, 49)]
SQ_ON_V = {0: True, 8: True, 16: True, 24: True, 32: True, 40: True, 48: True}

_CACHE = {}
LAST_RESULT = None
_OTILES = {}


def _build(has_bias=False, has_affine=False):
    import concourse.bass as bass
    import concourse.mybir as mybir
    from concourse.tile import TileContext

    fp32 = mybir.dt.float32
    bf16 = mybir.dt.bfloat16
    AF = mybir.ActivationFunctionType
    OP = mybir.AluOpType
    AX = mybir.AxisListType

    nc = bass.Bass()
    featTd = nc.declare_dram_parameter("featT", [128, 2, RPAD], bf16, isOutput=False)
    # wd cols: [0:256) Wr, [256:260) per-head col-sums of Wr, [260:268) A
    wd = nc.declare_dram_parameter("wd", [128, 2, 268], bf16, isOutput=False)
    mkd = nc.declare_dram_parameter("mkd", [128, NT * 4], fp32, isOutput=False)
    # general path consts: [0:256) gamma, [256:512) beta, row0 [512:768) br,
    # row0 [768:776) abias, row0 [776:904) ones
    exd = nc.declare_dram_parameter("exd", [128, 908], fp32, isOutput=False)
    outd = nc.declare_dram_parameter("out", [128, NT, 256], bf16, isOutput=True)

    # featT DMA chunks (rows) — finer early so the lv prepass starts sooner
    FCHUNKS = [(0, 1536), (1536, 3072), (3072, 4672), (4672, 6272)]

    with TileContext(nc) as tc:
        with (
            tc.tile_pool(name="const", bufs=1) as cpool,
            tc.tile_pool(name="work", bufs=1) as wpool,
            tc.tile_pool(name="p2", bufs=2) as p2pool,
            tc.tile_pool(name="stat", bufs=2) as stpool,
            tc.tile_pool(name="o", bufs=5) as opool,
            tc.tile_pool(name="y", bufs=3) as ypool,
            tc.tile_pool(name="sq", bufs=4) as sqpool,
            tc.tile_pool(name="psv", bufs=2, space="PSUM") as pvpool,
            tc.tile_pool(name="pslv", bufs=1, space="PSUM") as plpool,
        ):
            ft_sb = cpool.tile([128, 2, RPAD], bf16, tag="ft")
            w_sb = cpool.tile([128, 2, 268], bf16, tag="w")
            mk_sb = cpool.tile([128, NT, 4], fp32, tag="mk")
            ex_sb = cpool.tile([128, 908], fp32, tag="exd")
            warm = cpool.tile([128, 1], fp32, tag="warm")
            warmp = cpool.tile([128, 1], fp32, tag="warmp")
            epsc = cpool.tile([128, 1], fp32, tag="epsc")
            nc.gpsimd.memset(epsc[:], EPS)

            nc.gpsimd.dma_start(out=w_sb[:], in_=wd[:])
            nc.gpsimd.dma_start(
                out=mk_sb[:].rearrange("p t r -> p (t r)"), in_=mkd[:])
            if has_affine or has_bias:
                nc.gpsimd.dma_start(out=ex_sb[:], in_=exd[:])
            for (r0, r1) in FCHUNKS:
                nc.sync.dma_start(out=ft_sb[:, :, r0:r1],
                                  in_=featTd[:, :, r0:r1])

            # prewarm the single activation table set while DMAs run
            nc.scalar.activation(warm[:], epsc[:], AF.Exp)

            lv_ps = plpool.tile([128, NTU, 16], fp32, tag="lv")
            vs1_sb = wpool.tile([128, NTU, 4], fp32, tag="vs1")
            sso = wpool.tile([128, NTU], fp32, tag="sso")
            lvl_sb = wpool.tile([128, NTU, 4], fp32, tag="lvl")
            lvr_sb = wpool.tile([128, NTU, 4], fp32, tag="lvr")
            s4_sb = wpool.tile([128, NTU, 4], fp32, tag="s4")
            st6 = wpool.tile([128, NTU, 6], fp32, tag="st6")
            rstd = wpool.tile([128, NTU], fp32, tag="rstd")
            nb = wpool.tile([128, NTU], fp32, tag="nb")
            mu_sb = wpool.tile([128, NTU], fp32, tag="mu")

            def lv_prepass(t0, t1):
                for t in range(t0, t1):
                    r0 = t * 128
                    nc.tensor.matmul(lv_ps[:, t, 0:12], ft_sb[:, 0, r0:r0 + 128],
                                     w_sb[:, 0, 256:268], start=True,
                                     stop=not has_bias)
                    nc.tensor.matmul(lv_ps[:, t, 0:12], ft_sb[:, 1, r0:r0 + 128],
                                     w_sb[:, 1, 256:268], start=False,
                                     stop=not has_bias)
                    if has_bias:
                        nc.tensor.matmul(lv_ps[:, t, 0:12], ex_sb[0:1, 780:908],
                                         ex_sb[0:1, 768:780], start=False,
                                         stop=True)

            def phase2(ci):
                # layout [p, r, t, h] so every AP stays within 2 free dims
                t0, t1 = PH_CHUNKS[ci]
                tn = t1 - t0
                nc.scalar.copy(lvl_sb[:, t0:t1, :], lv_ps[:, t0:t1, 4:8])
                nc.scalar.copy(lvr_sb[:, t0:t1, :], lv_ps[:, t0:t1, 8:12])
                nc.scalar.copy(vs1_sb[:, t0:t1, :], lv_ps[:, t0:t1, 0:4])
                # vl/vr broadcast over r: [p, 1->4r, (t h)]
                vl3 = lvl_sb[:, t0:t1, :].rearrange("p t h -> p (t h)") \
                    .unsqueeze(1).broadcast_to((128, 4, tn * 4))
                vr3 = lvr_sb[:, t0:t1, :].rearrange("p t h -> p (t h)") \
                    .unsqueeze(1).broadcast_to((128, 4, tn * 4))
                # mask transposed to r-major then expanded over h
                mkT = p2pool.tile([128, 4, tn], fp32, tag="mkT", name="mkT")
                nc.scalar.copy(
                    mkT[:], mk_sb[:, t0:t1, :].rearrange("p t r -> p r t"))
                mkx = p2pool.tile([128, 4 * tn, 4], fp32, tag="mkx", name="mkx")
                nc.scalar.copy(
                    mkx[:],
                    mkT[:].rearrange("p r t -> p (r t)").unsqueeze(2)
                        .broadcast_to((128, 4 * tn, 4)))
                lg = p2pool.tile([128, 4, tn, 4], fp32, tag="lg", name="lg")
                lgs = lg[:].rearrange("p r t h -> p r (t h)")
                lgf = lg[:].rearrange("p r t h -> p (r t h)")
                nc.vector.scalar_tensor_tensor(
                    out=lgs,
                    in0=mkx[:].rearrange("p rt h -> p (rt h)")
                        .rearrange("p (r th) -> p r th", r=4),
                    scalar=1.0, in1=vr3, op0=OP.bypass, op1=OP.mult)
                nc.vector.tensor_tensor(out=lgs, in0=lgs, in1=vl3, op=OP.add)
                lr = p2pool.tile([128, 4, tn, 4], fp32, tag="lr", name="lr")
                lrf = lr[:].rearrange("p r t h -> p (r t h)")
                nc.vector.scalar_tensor_tensor(out=lrf, in0=lgf, scalar=0.2,
                                               in1=lgf, op0=OP.mult, op1=OP.max)
                ext = p2pool.tile([128, 4, tn, 4], fp32, tag="ext", name="ext")
                exf = ext[:].rearrange("p r t h -> p (r t h)")
                ex3 = ext[:].rearrange("p r t h -> p (r t) h")
                nc.scalar.activation(exf, lrf, AF.Exp)
                den = p2pool.tile([128, 4 * tn], fp32, tag="den", name="den")
                nc.vector.tensor_reduce(out=den[:], in_=ex3, axis=AX.X, op=OP.add)
                rden = p2pool.tile([128, 4 * tn], fp32, tag="rden", name="rden")
                nc.vector.reciprocal(rden[:], den[:])
                mrd = p2pool.tile([128, 4 * tn], fp32, tag="mrd", name="mrd")
                nc.vector.tensor_tensor(
                    out=mrd[:], in0=mkT[:].rearrange("p r t -> p (r t)"),
                    in1=rden[:], op=OP.mult)
                wex = p2pool.tile([128, 4, tn, 4], fp32, tag="wex", name="wex")
                wex3 = wex[:].rearrange("p r t h -> p (r t) h")
                nc.vector.scalar_tensor_tensor(
                    out=wex3, in0=ex3, scalar=1.0,
                    in1=mrd[:].unsqueeze(2).broadcast_to((128, 4 * tn, 4)),
                    op0=OP.bypass, op1=OP.mult)
                s4ab = p2pool.tile([128, 2, tn, 4], fp32, tag="s4ab", name="s4ab")
                s4abs = s4ab[:].rearrange("p r t h -> p r (t h)")
                nc.vector.tensor_tensor(
                    out=s4abs,
                    in0=wex[:, 0:2].rearrange("p r t h -> p r (t h)"),
                    in1=wex[:, 2:4].rearrange("p r t h -> p r (t h)"),
                    op=OP.add)
                nc.vector.tensor_tensor(
                    out=s4_sb[:, t0:t1, :].rearrange("p t h -> p (t h)"),
                    in0=s4ab[:, 0:1].rearrange("p r t h -> p (r t h)"),
                    in1=s4ab[:, 1:2].rearrange("p r t h -> p (r t h)"),
                    op=OP.add)

            def stats_chunk(c0, c1):
                tn = c1 - c0
                varm = stpool.tile([128, 8], fp32, tag="varm", name="varm")
                lnv = stpool.tile([128, 8], fp32, tag="lnv", name="lnv")
                if SQ_ON_V[c0]:
                    sl = st6[:, c0:c1, :]
                    m2 = stpool.tile([128, 8], fp32, tag="m2", name="m2")
                    dd = stpool.tile([128, 8], fp32, tag="dd", name="dd")
                    cv = stpool.tile([128, 8], fp32, tag="cv", name="cv")
                    d2 = stpool.tile([128, 8], fp32, tag="d2", name="d2")
                    nc.vector.tensor_tensor(out=m2[:, 0:tn], in0=sl[:, :, 1],
                                            in1=sl[:, :, 4], op=OP.add)
                    nc.vector.tensor_tensor(out=dd[:, 0:tn], in0=sl[:, :, 1],
                                            in1=sl[:, :, 4], op=OP.subtract)
                    nc.vector.tensor_tensor(out=cv[:, 0:tn], in0=sl[:, :, 2],
                                            in1=sl[:, :, 5], op=OP.add)
                    nc.vector.tensor_tensor(out=d2[:, 0:tn], in0=dd[:, 0:tn],
                                            in1=dd[:, 0:tn], op=OP.mult)
                    # var*256 = cv + 64*d2 ; mu = m2/2
                    nc.vector.scalar_tensor_tensor(
                        out=varm[:, 0:tn], in0=d2[:, 0:tn], scalar=64.0,
                        in1=cv[:, 0:tn], op0=OP.mult, op1=OP.add)
                    nc.vector.tensor_scalar_mul(mu_sb[:, c0:c1], m2[:, 0:tn],
                                                0.5)
                    nc.scalar.activation(lnv[:, 0:tn], varm[:, 0:tn], AF.Ln,
                                         scale=1.0 / 256.0, bias=epsc[:])
                else:
                    # mean from the hsum matmul columns; 2nd moment from the
                    # S-side Square accumulators (rescaled on S so the accum
                    # writes are ordered before V reads them)
                    ssos = stpool.tile([128, 8], fp32, tag="ssos", name="ssos")
                    nc.scalar.mul(ssos[:, 0:tn], sso[:, c0:c1], 1.0 / 256.0)
                    tmp = stpool.tile([128, 8, 4], fp32, tag="tmp", name="tmp")
                    mu256 = stpool.tile([128, 8], fp32, tag="mu256",
                                        name="mu256")
                    mu2 = stpool.tile([128, 8], fp32, tag="mu2", name="mu2")
                    nc.vector.tensor_tensor(out=tmp[:, 0:tn, :],
                                            in0=s4_sb[:, c0:c1, :],
                                            in1=vs1_sb[:, c0:c1, :],
                                            op=OP.mult)
                    nc.vector.tensor_reduce(out=mu256[:, 0:tn],
                                            in_=tmp[:, 0:tn, :], axis=AX.X,
                                            op=OP.add)
                    nc.vector.tensor_scalar_mul(mu_sb[:, c0:c1],
                                                mu256[:, 0:tn], 1.0 / 256.0)
                    nc.vector.tensor_tensor(out=mu2[:, 0:tn],
                                            in0=mu_sb[:, c0:c1],
                                            in1=mu_sb[:, c0:c1], op=OP.mult)
                    nc.vector.tensor_tensor(out=varm[:, 0:tn],
                                            in0=ssos[:, 0:tn],
                                            in1=mu2[:, 0:tn], op=OP.subtract)
                    nc.scalar.activation(lnv[:, 0:tn], varm[:, 0:tn], AF.Ln,
                                         scale=1.0, bias=epsc[:])
                nc.scalar.activation(rstd[:, c0:c1], lnv[:, 0:tn], AF.Exp,
                                     scale=-0.5)
                nc.vector.scalar_tensor_tensor(
                    out=nb[:, c0:c1], in0=mu_sb[:, c0:c1], scalar=-1.0,
                    in1=rstd[:, c0:c1], op0=OP.mult, op1=OP.mult)

            # ---- emission ----
            lv_prepass(0, 12)
            phase2(0)
            lv_prepass(12, 24)
            phase2(1)

            gam_bc = ex_sb[:, 0:256]
            bet_bc = ex_sb[:, 256:512]

            done_lvB = False
            for g, (tg, gn) in enumerate(G_TILES):
                if g == 2 and not done_lvB:
                    lv_prepass(24, NTU)
                    phase2(2)
                    done_lvB = True
                v_ps = pvpool.tile([128, 4, 256], fp32, tag="v", name="v_ps")
                for i in range(gn):
                    r0 = (tg + i) * 128
                    nc.tensor.matmul(v_ps[:, i, :], ft_sb[:, 0, r0:r0 + 128],
                                     w_sb[:, 0, 0:256], start=True, stop=False)
                    nc.tensor.matmul(v_ps[:, i, :], ft_sb[:, 1, r0:r0 + 128],
                                     w_sb[:, 1, 0:256], start=False,
                                     stop=not has_bias)
                    if has_bias:
                        nc.tensor.matmul(v_ps[:, i, 0:256], ex_sb[0:1, 780:908],
                                         ex_sb[0:1, 512:768], start=False,
                                         stop=True)
                o_t = opool.tile([128, 4, 256], bf16, tag="o", name="o_t")
                _OTILES[g] = o_t
                for hh in range(4):
                    nc.vector.scalar_tensor_tensor(
                        out=o_t[:, 0:gn, hh * 64:(hh + 1) * 64],
                        in0=v_ps[:, 0:gn, hh * 64:(hh + 1) * 64],
                        scalar=1.0,
                        in1=s4_sb[:, tg:tg + gn, hh:hh + 1]
                            .broadcast_to((128, gn, 64)),
                        op0=OP.bypass, op1=OP.mult)
                for i in range(gn):
                    t = tg + i
                    if SQ_ON_V[(t // 8) * 8]:
                        nc.vector.bn_stats(st6[:, t, :], o_t[:, i, :])
                    else:
                        sq_t = sqpool.tile([128, 256], bf16, tag="sq",
                                           name="sq_t")
                        nc.scalar.activation(sq_t[:], o_t[:, i, :], AF.Square,
                                             accum_out=sso[:, t:t + 1])

                # stats + F3 + out-DMA for every completed chunk
                for (c0, c1) in ST_CHUNKS:
                    if c1 == tg + gn:
                        stats_chunk(c0, c1)
                        for gg, (tg2, gn2) in enumerate(G_TILES):
                            if not (c0 <= tg2 < c1):
                                continue
                            y_t = ypool.tile([128, 4, 256], bf16, tag="y",
                                             name="y_t")
                            og = _OTILES[gg]
                            for i in range(gn2):
                                t = tg2 + i
                                if not has_affine and False:
                                    # V path: y = (max(o,mu)-mu)*rstd
                                    # == relu((o-mu)*rstd), rstd > 0
                                    t1 = ypool.tile([128, 256], bf16,
                                                    tag="t1", name="t1")
                                    nc.vector.tensor_scalar(
                                        out=t1[:], in0=og[:, i, :],
                                        scalar1=mu_sb[:, t:t + 1],
                                        scalar2=mu_sb[:, t:t + 1],
                                        op0=OP.max, op1=OP.subtract)
                                    nc.vector.tensor_scalar(
                                        out=y_t[:, i, :], in0=t1[:],
                                        scalar1=rstd[:, t:t + 1],
                                        scalar2=None, op0=OP.mult)
                                elif not has_affine:
                                    nc.scalar.activation(
                                        y_t[:, i, :], og[:, i, :], AF.Relu,
                                        scale=rstd[:, t:t + 1],
                                        bias=nb[:, t:t + 1])
                                else:
                                    z_t = ypool.tile([128, 256], fp32, tag="z",
                                                     name="z_t")
                                    nc.scalar.activation(
                                        z_t[:], og[:, i, :], AF.Identity,
                                        scale=rstd[:, t:t + 1],
                                        bias=nb[:, t:t + 1])
                                    gz = ypool.tile([128, 256], fp32, tag="gz",
                                                    name="gz")
                                    nc.vector.tensor_tensor(
                                        out=gz[:], in0=z_t[:], in1=gam_bc,
                                        op=OP.mult)
                                    zb = ypool.tile([128, 256], fp32, tag="zb",
                                                    name="zb")
                                    nc.vector.tensor_tensor(
                                        out=zb[:], in0=gz[:], in1=bet_bc,
                                        op=OP.add)
                                    nc.scalar.activation(y_t[:, i, :], zb[:],
                                                         AF.Relu)
                            nc.sync.dma_start(
                                out=outd[:, tg2:tg2 + gn2, :],
                                in_=y_t[:, 0:gn2, :])

    return nc


def _split_waits(bir_bytes):
    """Walrus on this stack only accepts one sync-wait per instruction.
    Split extra waits into standalone single-wait NoOps on the same
    engine queue (exact raw-bass semantics: in-order queue stalls)."""
    import orjson
    m = orjson.loads(bir_bytes)
    counter = [0]

    def proc(obj):
        if isinstance(obj, dict):
            for k, v in obj.items():
                if k == "instructions" and isinstance(v, list):
                    new = []
                    for ins in v:
                        si = ins.get("sync_info")
                        waits = (si or {}).get("on_wait") or []
                        lim = 0 if ins.get("opcode") == "ISA" else 1
                        if si and len(waits) > lim:
                            keep = waits[-lim:] if lim else []
                            for w in (waits[:-1] if lim else waits):
                                counter[0] += 1
                                new.append({
                                    "name": f"I-wsplit-{counter[0]}",
                                    "opcode": "EventSemaphore",
                                    "engine": ins.get("engine"),
                                    "ins": [], "outs": [],
                                    "debug": ins.get("debug"),
                                    "sync_info": {"on_update": [],
                                                  "on_wait": [w]},
                                })
                            si["on_wait"] = keep
                        new.append(ins)
                        proc(ins)
                    obj[k] = new
                else:
                    proc(v)
        elif isinstance(obj, list):
            for x in obj:
                proc(x)

    proc(m)
    return orjson.dumps(m)


def kernel(**inputs):
    global LAST_RESULT
    import os
    import ml_dtypes
    from concourse.bass_utils import run_bass_kernel_spmd

    feat = np.asarray(inputs["feat"], dtype=np.float32)
    Wr = np.asarray(inputs["Wr"], dtype=np.float32)
    br = np.asarray(inputs["br"], dtype=np.float32)
    rl = np.asarray(inputs["rel_attn_l"], dtype=np.float32)
    rr = np.asarray(inputs["rel_attn_r"], dtype=np.float32)
    g = np.asarray(inputs["ln_gamma"], dtype=np.float32)
    b = np.asarray(inputs["ln_beta"], dtype=np.float32)

    mask = np.ones((N, 4), np.float32)
    for m in range(M):
        dst = np.asarray(inputs[f"dst{m}"])
        mask[:, m] = np.bincount(dst, minlength=N) > 0

    # fold rel_attn into the weight matrix: vl = feat @ (Wr @ RLbd) (+br terms)
    rl_bd = np.zeros((256, 4), np.float32)
    rr_bd = np.zeros((256, 4), np.float32)
    for h in range(H):
        rl_bd[h * C:(h + 1) * C, h] = rl[h]
        rr_bd[h * C:(h + 1) * C, h] = rr[h]
    A = np.concatenate([Wr @ rl_bd, Wr @ rr_bd], axis=1)          # [256, 8]
    abias = np.concatenate([br @ rl_bd, br @ rr_bd])              # [8]
    hsum = Wr.reshape(256, 4, 64).sum(axis=2)                     # [256, 4]

    has_bias = bool(np.abs(br).max() > 0)
    has_affine = bool(np.abs(g - 1.0).max() > 0 or np.abs(b).max() > 0)

    Wfull = np.concatenate([Wr, hsum, A], axis=1)                 # [256, 268]
    # wd[p, kc, j] = Wfull[kc*128 + p, j]
    w_host = np.ascontiguousarray(
        Wfull.reshape(2, 128, 268).transpose(1, 0, 2)).astype(ml_dtypes.bfloat16)

    exd = np.zeros((128, 908), np.float32)
    exd[:, 0:256] = g
    exd[:, 256:512] = b
    exd[0, 512:768] = br
    exd[0, 768:772] = br.reshape(4, 64).sum(axis=1)
    exd[0, 772:780] = abias
    exd[0, 780:908] = 1.0

    key = (has_bias, has_affine)
    if key not in _CACHE:
        _OTILES.clear()
        nc0 = _build(has_bias=has_bias, has_affine=has_affine)
        _orig = nc0.to_json_bytes
        nc0.to_json_bytes = lambda: _split_waits(_orig())
        _CACHE[key] = nc0
    nc = _CACHE[key]

    in_maps = []
    for s in range(NCORES):
        fs = np.zeros((RPAD, 256), np.float32)
        fs[:RPC] = feat[s * RPC:(s + 1) * RPC]
        # featT[p, kc, j] = fs[j, kc*128 + p]
        ftT = np.ascontiguousarray(
            fs.T.reshape(2, 128, RPAD).transpose(1, 0, 2)).astype(ml_dtypes.bfloat16)
        mk = np.ones((RPAD, 4), np.float32)
        mk[:RPC] = mask[s * RPC:(s + 1) * RPC]
        mkh = np.ascontiguousarray(
            mk.reshape(NT, 128, 4).transpose(1, 0, 2)).reshape(128, NT * 4)
        in_maps.append({"featT": ftT, "wd": w_host, "mkd": mkh, "exd": exd})

    trace = bool(int(os.environ.get("KERNEL_TRACE", "0")))
    res = run_bass_kernel_spmd(nc, in_maps, list(range(NCORES)), trace=trace)
    LAST_RESULT = res
    outs = []
    for s in range(NCORES):
        o = np.asarray(res.results[s]["out"]).astype(np.float32)  # [128, NT, 256]
        outs.append(o.transpose(1, 0, 2).reshape(RPAD, 256)[:RPC])
    return np.concatenate(outs, axis=0)


# revision 49
# speedup vs baseline: 1.2264x; 1.0123x over previous
"""LATTE GNN forward on 8 Trainium2 NeuronCores — v3.

Math (same collapse as baseline): per-edge message is v[dst], and the
segment-softmax weights over each dst's incoming edges sum to 1, so
    h_m[n] = v[n] * mask_m[n],  mask_m[n] = [n has an incoming edge in rel m]
    v      = feat @ Wr + br
    vl[n,h] = v[n,h,:].rel_attn_l[h]   (folded: feat @ (Wr @ RLbd))
    vr[n,h] = v[n,h,:].rel_attn_r[h]
    logit[n,r,h] = lrelu(vl + mask_r*vr);  beta = softmax over h (per r)
    s[n,h] = sum_r mask_r[n] * beta[n,r,h]   (mask_3 = 1)
    y      = relu(LN(v * s) * gamma + ln_beta)

v3 engine plan (372us baseline -> 78us v2 -> this):
  - bf16 matmuls, whole featT resident in SBUF (6 chunked DMAs)
  - lv = feat@A in a tiny PE pre-pass into one persistent PSUM bank
  - softmax chain batched over 24/25-tile chunks on V (+one big exp on S)
  - LN mean via 4 extra matmul cols (per-head column sums of Wr);
    LN second moment via GpSimd square+accum per tile (Pool engine,
    otherwise idle); no bn_stats on V
  - rstd = exp(-0.5*ln(var+eps)) so S only ever uses the
    natural_log_exp table set -> ONE ACT_TABLE_LOAD total
  - LN tail fused to one S op per tile: y = Relu(rstd*o - mu*rstd), bf16 out
  - only 49 tiles computed (50048 rows >= 6250 real rows/core)
Node-sharded 6250 rows/core, padded to 6656 = 52*128 (3 pad tiles skipped).
"""

import numpy as np

N, D, H, C, M = 50000, 256, 4, 64, 3
NCORES = 8
RPC = N // NCORES          # 6250 rows per core
NT = 52                    # tile slots in dram layout
NTU = 49                   # tiles actually computed (49*128 = 6272 >= 6250)
RPAD = NT * 128            # 6656
EPS = 1e-5
G_TILES = [(g * 4, 4) for g in range(12)] + [(48, 1)]
PH_CHUNKS = [(0, 12), (12, 24), (24, 49)]       # phase-2 tile ranges
ST_CHUNKS = [(0, 8), (8, 16), (16, 24), (24, 32), (32, 40), (40, 48), (48, 49)]
SQ_ON_V = {0: True, 8: True, 16: True, 24: True, 32: True, 40: True, 48: True}

_CACHE = {}
LAST_RESULT = None
_OTILES = {}


def _build(has_bias=False, has_affine=False):
    import concourse.bass as bass
    import concourse.mybir as mybir
    from concourse.tile import TileContext

    fp32 = mybir.dt.float32
    bf16 = mybir.dt.bfloat16
    AF = mybir.ActivationFunctionType
    OP = mybir.AluOpType
    AX = mybir.AxisListType

    nc = bass.Bass()
    featTd = nc.declare_dram_parameter("featT", [128, 2, RPAD], bf16, isOutput=False)
    # wd cols: [0:256) Wr, [256:260) per-head col-sums of Wr, [260:268) A
    wd = nc.declare_dram_parameter("wd", [128, 2, 268], bf16, isOutput=False)
    mkd = nc.declare_dram_parameter("mkd", [128, NT * 4], fp32, isOutput=False)
    # general path consts: [0:256) gamma, [256:512) beta, row0 [512:768) br,
    # row0 [768:776) abias, row0 [776:904) ones
    exd = nc.declare_dram_parameter("exd", [128, 908], fp32, isOutput=False)
    outd = nc.declare_dram_parameter("out", [128, NT, 256], bf16, isOutput=True)

    # featT DMA chunks (rows) — finer early so the lv prepass starts sooner
    FCHUNKS = [(0, 768), (768, 1536), (1536, 3072), (3072, 4672), (4672, 6272)]

    with TileContext(nc) as tc:
        with (
            tc.tile_pool(name="const", bufs=1) as cpool,
            tc.tile_pool(name="work", bufs=1) as wpool,
            tc.tile_pool(name="p2", bufs=2) as p2pool,
            tc.tile_pool(name="stat", bufs=2) as stpool,
            tc.tile_pool(name="o", bufs=6) as opool,
            tc.tile_pool(name="y", bufs=3) as ypool,
            tc.tile_pool(name="sq", bufs=4) as sqpool,
            tc.tile_pool(name="psv", bufs=3, space="PSUM") as pvpool,
            tc.tile_pool(name="pslv", bufs=1, space="PSUM") as plpool,
        ):
            ft_sb = cpool.tile([128, 2, RPAD], bf16, tag="ft")
            w_sb = cpool.tile([128, 2, 268], bf16, tag="w")
            mk_sb = cpool.tile([128, NT, 4], fp32, tag="mk")
            ex_sb = cpool.tile([128, 908], fp32, tag="exd")
            warm = cpool.tile([128, 1], fp32, tag="warm")
            warmp = cpool.tile([128, 1], fp32, tag="warmp")
            epsc = cpool.tile([128, 1], fp32, tag="epsc")
            nc.gpsimd.memset(epsc[:], EPS)

            nc.gpsimd.dma_start(out=w_sb[:], in_=wd[:])
            nc.gpsimd.dma_start(
                out=mk_sb[:].rearrange("p t r -> p (t r)"), in_=mkd[:])
            if has_affine or has_bias:
                nc.gpsimd.dma_start(out=ex_sb[:], in_=exd[:])
            for (r0, r1) in FCHUNKS:
                nc.sync.dma_start(out=ft_sb[:, :, r0:r1],
                                  in_=featTd[:, :, r0:r1])

            # prewarm the single activation table set while DMAs run
            nc.scalar.activation(warm[:], epsc[:], AF.Exp)

            lv_ps = plpool.tile([128, NTU, 16], fp32, tag="lv")
            vs1_sb = wpool.tile([128, NTU, 4], fp32, tag="vs1")
            sso = wpool.tile([128, NTU], fp32, tag="sso")
            lvl_sb = wpool.tile([128, NTU, 4], fp32, tag="lvl")
            lvr_sb = wpool.tile([128, NTU, 4], fp32, tag="lvr")
            s4_sb = wpool.tile([128, NTU, 4], fp32, tag="s4")
            st6 = wpool.tile([128, NTU, 6], fp32, tag="st6")
            rstd = wpool.tile([128, NTU], fp32, tag="rstd")
            nb = wpool.tile([128, NTU], fp32, tag="nb")
            mu_sb = wpool.tile([128, NTU], fp32, tag="mu")

            def lv_prepass(t0, t1):
                for t in range(t0, t1):
                    r0 = t * 128
                    nc.tensor.matmul(lv_ps[:, t, 0:12], ft_sb[:, 0, r0:r0 + 128],
                                     w_sb[:, 0, 256:268], start=True,
                                     stop=not has_bias)
                    nc.tensor.matmul(lv_ps[:, t, 0:12], ft_sb[:, 1, r0:r0 + 128],
                                     w_sb[:, 1, 256:268], start=False,
                                     stop=not has_bias)
                    if has_bias:
                        nc.tensor.matmul(lv_ps[:, t, 0:12], ex_sb[0:1, 780:908],
                                         ex_sb[0:1, 768:780], start=False,
                                         stop=True)

            def phase2(ci):
                # layout [p, r, t, h] so every AP stays within 2 free dims
                t0, t1 = PH_CHUNKS[ci]
                tn = t1 - t0
                nc.scalar.copy(lvl_sb[:, t0:t1, :], lv_ps[:, t0:t1, 4:8])
                nc.scalar.copy(lvr_sb[:, t0:t1, :], lv_ps[:, t0:t1, 8:12])
                nc.scalar.copy(vs1_sb[:, t0:t1, :], lv_ps[:, t0:t1, 0:4])
                # vl/vr broadcast over r: [p, 1->4r, (t h)]
                vl3 = lvl_sb[:, t0:t1, :].rearrange("p t h -> p (t h)") \
                    .unsqueeze(1).broadcast_to((128, 4, tn * 4))
                vr3 = lvr_sb[:, t0:t1, :].rearrange("p t h -> p (t h)") \
                    .unsqueeze(1).broadcast_to((128, 4, tn * 4))
                # mask transposed to r-major then expanded over h
                mkT = p2pool.tile([128, 4, tn], fp32, tag="mkT", name="mkT")
                nc.scalar.copy(
                    mkT[:], mk_sb[:, t0:t1, :].rearrange("p t r -> p r t"))
                mkx = p2pool.tile([128, 4 * tn, 4], fp32, tag="mkx", name="mkx")
                nc.scalar.copy(
                    mkx[:],
                    mkT[:].rearrange("p r t -> p (r t)").unsqueeze(2)
                        .broadcast_to((128, 4 * tn, 4)))
                lg = p2pool.tile([128, 4, tn, 4], fp32, tag="lg", name="lg")
                lgs = lg[:].rearrange("p r t h -> p r (t h)")
                lgf = lg[:].rearrange("p r t h -> p (r t h)")
                nc.vector.scalar_tensor_tensor(
                    out=lgs,
                    in0=mkx[:].rearrange("p rt h -> p (rt h)")
                        .rearrange("p (r th) -> p r th", r=4),
                    scalar=1.0, in1=vr3, op0=OP.bypass, op1=OP.mult)
                nc.vector.tensor_tensor(out=lgs, in0=lgs, in1=vl3, op=OP.add)
                lr = p2pool.tile([128, 4, tn, 4], fp32, tag="lr", name="lr")
                lrf = lr[:].rearrange("p r t h -> p (r t h)")
                nc.vector.scalar_tensor_tensor(out=lrf, in0=lgf, scalar=0.2,
                                               in1=lgf, op0=OP.mult, op1=OP.max)
                ext = p2pool.tile([128, 4, tn, 4], fp32, tag="ext", name="ext")
                exf = ext[:].rearrange("p r t h -> p (r t h)")
                ex3 = ext[:].rearrange("p r t h -> p (r t) h")
                nc.scalar.activation(exf, lrf, AF.Exp)
                den = p2pool.tile([128, 4 * tn], fp32, tag="den", name="den")
                nc.vector.tensor_reduce(out=den[:], in_=ex3, axis=AX.X, op=OP.add)
                rden = p2pool.tile([128, 4 * tn], fp32, tag="rden", name="rden")
                nc.vector.reciprocal(rden[:], den[:])
                mrd = p2pool.tile([128, 4 * tn], fp32, tag="mrd", name="mrd")
                nc.vector.tensor_tensor(
                    out=mrd[:], in0=mkT[:].rearrange("p r t -> p (r t)"),
                    in1=rden[:], op=OP.mult)
                wex = p2pool.tile([128, 4, tn, 4], fp32, tag="wex", name="wex")
                wex3 = wex[:].rearrange("p r t h -> p (r t) h")
                nc.vector.scalar_tensor_tensor(
                    out=wex3, in0=ex3, scalar=1.0,
                    in1=mrd[:].unsqueeze(2).broadcast_to((128, 4 * tn, 4)),
                    op0=OP.bypass, op1=OP.mult)
                s4ab = p2pool.tile([128, 2, tn, 4], fp32, tag="s4ab", name="s4ab")
                s4abs = s4ab[:].rearrange("p r t h -> p r (t h)")
                nc.vector.tensor_tensor(
                    out=s4abs,
                    in0=wex[:, 0:2].rearrange("p r t h -> p r (t h)"),
                    in1=wex[:, 2:4].rearrange("p r t h -> p r (t h)"),
                    op=OP.add)
                nc.vector.tensor_tensor(
                    out=s4_sb[:, t0:t1, :].rearrange("p t h -> p (t h)"),
                    in0=s4ab[:, 0:1].rearrange("p r t h -> p (r t h)"),
                    in1=s4ab[:, 1:2].rearrange("p r t h -> p (r t h)"),
                    op=OP.add)

            def stats_chunk(c0, c1):
                tn = c1 - c0
                varm = stpool.tile([128, 8], fp32, tag="varm", name="varm")
                lnv = stpool.tile([128, 8], fp32, tag="lnv", name="lnv")
                if SQ_ON_V[c0]:
                    sl = st6[:, c0:c1, :]
                    m2 = stpool.tile([128, 8], fp32, tag="m2", name="m2")
                    dd = stpool.tile([128, 8], fp32, tag="dd", name="dd")
                    cv = stpool.tile([128, 8], fp32, tag="cv", name="cv")
                    d2 = stpool.tile([128, 8], fp32, tag="d2", name="d2")
                    nc.vector.tensor_tensor(out=m2[:, 0:tn], in0=sl[:, :, 1],
                                            in1=sl[:, :, 4], op=OP.add)
                    nc.vector.tensor_tensor(out=dd[:, 0:tn], in0=sl[:, :, 1],
                                            in1=sl[:, :, 4], op=OP.subtract)
                    nc.vector.tensor_tensor(out=cv[:, 0:tn], in0=sl[:, :, 2],
                                            in1=sl[:, :, 5], op=OP.add)
                    nc.vector.tensor_tensor(out=d2[:, 0:tn], in0=dd[:, 0:tn],
                                            in1=dd[:, 0:tn], op=OP.mult)
                    # var*256 = cv + 64*d2 ; mu = m2/2
                    nc.vector.scalar_tensor_tensor(
                        out=varm[:, 0:tn], in0=d2[:, 0:tn], scalar=64.0,
                        in1=cv[:, 0:tn], op0=OP.mult, op1=OP.add)
                    nc.vector.tensor_scalar_mul(mu_sb[:, c0:c1], m2[:, 0:tn],
                                                0.5)
                    nc.scalar.activation(lnv[:, 0:tn], varm[:, 0:tn], AF.Ln,
                                         scale=1.0 / 256.0, bias=epsc[:])
                else:
                    # mean from the hsum matmul columns; 2nd moment from the
                    # S-side Square accumulators (rescaled on S so the accum
                    # writes are ordered before V reads them)
                    ssos = stpool.tile([128, 8], fp32, tag="ssos", name="ssos")
                    nc.scalar.mul(ssos[:, 0:tn], sso[:, c0:c1], 1.0 / 256.0)
                    tmp = stpool.tile([128, 8, 4], fp32, tag="tmp", name="tmp")
                    mu256 = stpool.tile([128, 8], fp32, tag="mu256",
                                        name="mu256")
                    mu2 = stpool.tile([128, 8], fp32, tag="mu2", name="mu2")
                    nc.vector.tensor_tensor(out=tmp[:, 0:tn, :],
                                            in0=s4_sb[:, c0:c1, :],
                                            in1=vs1_sb[:, c0:c1, :],
                                            op=OP.mult)
                    nc.vector.tensor_reduce(out=mu256[:, 0:tn],
                                            in_=tmp[:, 0:tn, :], axis=AX.X,
                                            op=OP.add)
                    nc.vector.tensor_scalar_mul(mu_sb[:, c0:c1],
                                                mu256[:, 0:tn], 1.0 / 256.0)
                    nc.vector.tensor_tensor(out=mu2[:, 0:tn],
                                            in0=mu_sb[:, c0:c1],
                                            in1=mu_sb[:, c0:c1], op=OP.mult)
                    nc.vector.tensor_tensor(out=varm[:, 0:tn],
                                            in0=ssos[:, 0:tn],
                                            in1=mu2[:, 0:tn], op=OP.subtract)
                    nc.scalar.activation(lnv[:, 0:tn], varm[:, 0:tn], AF.Ln,
                                         scale=1.0, bias=epsc[:])
                nc.scalar.activation(rstd[:, c0:c1], lnv[:, 0:tn], AF.Exp,
                                     scale=-0.5)
                nc.vector.scalar_tensor_tensor(
                    out=nb[:, c0:c1], in0=mu_sb[:, c0:c1], scalar=-1.0,
                    in1=rstd[:, c0:c1], op0=OP.mult, op1=OP.mult)

            # ---- emission ----
            lv_prepass(0, 12)
            phase2(0)
            lv_prepass(12, 24)
            phase2(1)

            gam_bc = ex_sb[:, 0:256]
            bet_bc = ex_sb[:, 256:512]

            done_lvB = False
            for g, (tg, gn) in enumerate(G_TILES):
                if g == 2 and not done_lvB:
                    lv_prepass(24, NTU)
                    phase2(2)
                    done_lvB = True
                v_ps = pvpool.tile([128, 4, 256], fp32, tag="v", name="v_ps")
                for i in range(gn):
                    r0 = (tg + i) * 128
                    nc.tensor.matmul(v_ps[:, i, :], ft_sb[:, 0, r0:r0 + 128],
                                     w_sb[:, 0, 0:256], start=True, stop=False)
                    nc.tensor.matmul(v_ps[:, i, :], ft_sb[:, 1, r0:r0 + 128],
                                     w_sb[:, 1, 0:256], start=False,
                                     stop=not has_bias)
                    if has_bias:
                        nc.tensor.matmul(v_ps[:, i, 0:256], ex_sb[0:1, 780:908],
                                         ex_sb[0:1, 512:768], start=False,
                                         stop=True)
                o_t = opool.tile([128, 4, 256], bf16, tag="o", name="o_t")
                _OTILES[g] = o_t
                for hh in range(4):
                    nc.vector.scalar_tensor_tensor(
                        out=o_t[:, 0:gn, hh * 64:(hh + 1) * 64],
                        in0=v_ps[:, 0:gn, hh * 64:(hh + 1) * 64],
                        scalar=1.0,
                        in1=s4_sb[:, tg:tg + gn, hh:hh + 1]
                            .broadcast_to((128, gn, 64)),
                        op0=OP.bypass, op1=OP.mult)
                for i in range(gn):
                    t = tg + i
                    if SQ_ON_V[(t // 8) * 8]:
                        nc.vector.bn_stats(st6[:, t, :], o_t[:, i, :])
                    else:
                        sq_t = sqpool.tile([128, 256], bf16, tag="sq",
                                           name="sq_t")
                        nc.scalar.activation(sq_t[:], o_t[:, i, :], AF.Square,
                                             accum_out=sso[:, t:t + 1])

                # stats + F3 + out-DMA for every completed chunk
                for (c0, c1) in ST_CHUNKS:
                    if c1 == tg + gn:
                        stats_chunk(c0, c1)
                        for gg, (tg2, gn2) in enumerate(G_TILES):
                            if not (c0 <= tg2 < c1):
                                continue
                            y_t = ypool.tile([128, 4, 256], bf16, tag="y",
                                             name="y_t")
                            og = _OTILES[gg]
                            for i in range(gn2):
                                t = tg2 + i
                                if not has_affine and False:
                                    # V path: y = (max(o,mu)-mu)*rstd
                                    # == relu((o-mu)*rstd), rstd > 0
                                    t1 = ypool.tile([128, 256], bf16,
                                                    tag="t1", name="t1")
                                    nc.vector.tensor_scalar(
                                        out=t1[:], in0=og[:, i, :],
                                        scalar1=mu_sb[:, t:t + 1],
                                        scalar2=mu_sb[:, t:t + 1],
                                        op0=OP.max, op1=OP.subtract)
                                    nc.vector.tensor_scalar(
                                        out=y_t[:, i, :], in0=t1[:],
                                        scalar1=rstd[:, t:t + 1],
                                        scalar2=None, op0=OP.mult)
                                elif not has_affine:
                                    nc.scalar.activation(
                                        y_t[:, i, :], og[:, i, :], AF.Relu,
                                        scale=rstd[:, t:t + 1],
                                        bias=nb[:, t:t + 1])
                                else:
                                    z_t = ypool.tile([128, 256], fp32, tag="z",
                                                     name="z_t")
                                    nc.scalar.activation(
                                        z_t[:], og[:, i, :], AF.Identity,
                                        scale=rstd[:, t:t + 1],
                                        bias=nb[:, t:t + 1])
                                    gz = ypool.tile([128, 256], fp32, tag="gz",
                                                    name="gz")
                                    nc.vector.tensor_tensor(
                                        out=gz[:], in0=z_t[:], in1=gam_bc,
                                        op=OP.mult)
                                    zb = ypool.tile([128, 256], fp32, tag="zb",
                                                    name="zb")
                                    nc.vector.tensor_tensor(
                                        out=zb[:], in0=gz[:], in1=bet_bc,
                                        op=OP.add)
                                    nc.scalar.activation(y_t[:, i, :], zb[:],
                                                         AF.Relu)
                            nc.sync.dma_start(
                                out=outd[:, tg2:tg2 + gn2, :],
                                in_=y_t[:, 0:gn2, :])

    return nc


def _split_waits(bir_bytes):
    """Walrus on this stack only accepts one sync-wait per instruction.
    Split extra waits into standalone single-wait NoOps on the same
    engine queue (exact raw-bass semantics: in-order queue stalls)."""
    import orjson
    m = orjson.loads(bir_bytes)
    counter = [0]

    def proc(obj):
        if isinstance(obj, dict):
            for k, v in obj.items():
                if k == "instructions" and isinstance(v, list):
                    new = []
                    for ins in v:
                        si = ins.get("sync_info")
                        waits = (si or {}).get("on_wait") or []
                        lim = 0 if ins.get("opcode") == "ISA" else 1
                        if si and len(waits) > lim:
                            keep = waits[-lim:] if lim else []
                            for w in (waits[:-1] if lim else waits):
                                counter[0] += 1
                                new.append({
                                    "name": f"I-wsplit-{counter[0]}",
                                    "opcode": "EventSemaphore",
                                    "engine": ins.get("engine"),
                                    "ins": [], "outs": [],
                                    "debug": ins.get("debug"),
                                    "sync_info": {"on_update": [],
                                                  "on_wait": [w]},
                                })
                            si["on_wait"] = keep
                        new.append(ins)
                        proc(ins)
                    obj[k] = new
                else:
                    proc(v)
        elif isinstance(obj, list):
            for x in obj:
                proc(x)

    proc(m)
    return orjson.dumps(m)


def kernel(**inputs):
    global LAST_RESULT
    import os
    import ml_dtypes
    from concourse.bass_utils import run_bass_kernel_spmd

    feat = np.asarray(inputs["feat"], dtype=np.float32)
    Wr = np.asarray(inputs["Wr"], dtype=np.float32)
    br = np.asarray(inputs["br"], dtype=np.float32)
    rl = np.asarray(inputs["rel_attn_l"], dtype=np.float32)
    rr = np.asarray(inputs["rel_attn_r"], dtype=np.float32)
    g = np.asarray(inputs["ln_gamma"], dtype=np.float32)
    b = np.asarray(inputs["ln_beta"], dtype=np.float32)

    mask = np.ones((N, 4), np.float32)
    for m in range(M):
        dst = np.asarray(inputs[f"dst{m}"])
        mask[:, m] = np.bincount(dst, minlength=N) > 0

    # fold rel_attn into the weight matrix: vl = feat @ (Wr @ RLbd) (+br terms)
    rl_bd = np.zeros((256, 4), np.float32)
    rr_bd = np.zeros((256, 4), np.float32)
    for h in range(H):
        rl_bd[h * C:(h + 1) * C, h] = rl[h]
        rr_bd[h * C:(h + 1) * C, h] = rr[h]
    A = np.concatenate([Wr @ rl_bd, Wr @ rr_bd], axis=1)          # [256, 8]
    abias = np.concatenate([br @ rl_bd, br @ rr_bd])              # [8]
    hsum = Wr.reshape(256, 4, 64).sum(axis=2)                     # [256, 4]

    has_bias = bool(np.abs(br).max() > 0)
    has_affine = bool(np.abs(g - 1.0).max() > 0 or np.abs(b).max() > 0)

    Wfull = np.concatenate([Wr, hsum, A], axis=1)                 # [256, 268]
    # wd[p, kc, j] = Wfull[kc*128 + p, j]
    w_host = np.ascontiguousarray(
        Wfull.reshape(2, 128, 268).transpose(1, 0, 2)).astype(ml_dtypes.bfloat16)

    exd = np.zeros((128, 908), np.float32)
    exd[:, 0:256] = g
    exd[:, 256:512] = b
    exd[0, 512:768] = br
    exd[0, 768:772] = br.reshape(4, 64).sum(axis=1)
    exd[0, 772:780] = abias
    exd[0, 780:908] = 1.0

    key = (has_bias, has_affine)
    if key not in _CACHE:
        _OTILES.clear()
        nc0 = _build(has_bias=has_bias, has_affine=has_affine)
        _orig = nc0.to_json_bytes
        nc0.to_json_bytes = lambda: _split_waits(_orig())
        _CACHE[key] = nc0
    nc = _CACHE[key]

    in_maps = []
    for s in range(NCORES):
        fs = np.zeros((RPAD, 256), np.float32)
        fs[:RPC] = feat[s * RPC:(s + 1) * RPC]
        # featT[p, kc, j] = fs[j, kc*128 + p]
        ftT = np.ascontiguousarray(
            fs.T.reshape(2, 128, RPAD).transpose(1, 0, 2)).astype(ml_dtypes.bfloat16)
        mk = np.ones((RPAD, 4), np.float32)
        mk[:RPC] = mask[s * RPC:(s + 1) * RPC]
        mkh = np.ascontiguousarray(
            mk.reshape(NT, 128, 4).transpose(1, 0, 2)).reshape(128, NT * 4)
        in_maps.append({"featT": ftT, "wd": w_host, "mkd": mkh, "exd": exd})

    trace = bool(int(os.environ.get("KERNEL_TRACE", "0")))
    res = run_bass_kernel_spmd(nc, in_maps, list(range(NCORES)), trace=trace)
    LAST_RESULT = res
    outs = []
    for s in range(NCORES):
        o = np.asarray(res.results[s]["out"]).astype(np.float32)  # [128, NT, 256]
        outs.append(o.transpose(1, 0, 2).reshape(RPAD, 256)[:RPC])
    return np.concatenate(outs, axis=0)
